# revision 20
# baseline (speedup 1.0000x reference)
"""Distributed GCN (3x GCNConv + MLP classifier) on 8 Trainium2 NeuronCores, v2.

Strategy (graph/data parallel, dst-partitioned):
  - nodes are partitioned into 8 contiguous chunks (one per core), padded to a
    multiple of 128; real edges (no self-loops) are assigned to the core
    owning their dst node and bucketed by dst block of 128; self-loop
    contributions are applied algebraically in the epilogue.
  - phase A folds LayerNorm through the first matmul via linearity:
    z = rstd*(x @ Wf) - rstd*mu*colsum(Wf) + zb1, so no elementwise normalize
    pass is needed; x is supplied in bf16 twice (node-major for stats,
    feature-major for the PE matmul).
  - per layer: AllGather the dis-scaled z-table (bf16, 256B row stride,
    Shared-output collective), dma_gather z[src] rows per 128-edge tile on two
    SWDGE queues, aggregate agg[dst] += onehot(dst_rel).T @ z_rows on the
    TensorEngine (PSUM f32), batched epilogue across all blocks.
All heavy math runs on device; the host only preprocesses integer graph
structure, folds parameters, and slices/concatenates per-core arrays.
"""
import sys

for _p in ("/opt/trn_rl_repo",):
    if _p not in sys.path:
        sys.path.insert(0, _p)

import numpy as np
import ml_dtypes

import concourse.bass as bass
import concourse.bacc as bacc
import concourse.tile as tile
import concourse.mybir as mybir
from concourse import bass_utils

BF16 = ml_dtypes.bfloat16
F32 = mybir.dt.float32
BF = mybir.dt.bfloat16
EPS = 1e-5
NC = 8
P = 128          # partitions / block size
TS = 128         # z-table row stride in bf16 elements (256B, dma_gather req)
GIDX = 8192      # max gather indices per dma_gather call
NQ = 2           # SWDGE queues for gather round-robin


def _patch_dma_gather():
    """Relax bass's elem_size%256B assert: the Q7 ucode only needs the row
    STRIDE to be a multiple of 256B; the payload can be narrower."""
    import inspect, re, textwrap
    import concourse.bass as cbass
    src = textwrap.dedent(inspect.getsource(cbass.BassGpSimd.dma_gather))
    if "elem_size_bytes > 0 and elem_size_bytes % 256 == 0" not in src:
        return  # already patched
    pat = re.compile(
        r"assert \(\s*elem_size_bytes > 0 and elem_size_bytes % 256 == 0\s*\)"
        r"\s*#[^\n]*", re.S)
    assert pat.search(src), "dma_gather source changed; update patch"
    src = pat.sub("assert elem_size_bytes > 0", src)
    ns = vars(cbass).copy()
    exec(compile(src, "<patched_dma_gather>", "exec"), ns)
    cbass.BassGpSimd.dma_gather = ns["dma_gather"]


_patch_dma_gather()


def _to_bf(a):
    return np.ascontiguousarray(np.asarray(a, np.float32)).astype(BF16)


def _rep(v):
    """Replicate a 1-D param across 128 partitions -> [128, len]."""
    v = np.asarray(v, np.float32).reshape(1, -1)
    return np.ascontiguousarray(np.repeat(v, P, 0))


def preprocess(x, edge_index, ln_g, ln_b, W1, b1, bn1_g, bn1_b, bn1_m, bn1_v,
               W2, b2, bn2_g, bn2_b, bn2_m, bn2_v, W3, b3, bn3_g, bn3_b, bn3_m,
               bn3_v, fc1_W, fc1_b, lnc_g, lnc_b, fc2_W, fc2_b):
    N, D = x.shape
    E = edge_index.shape[1]
    H1, H2, H3 = W1.shape[1], W2.shape[1], W3.shape[1]
    HC, C = fc1_W.shape[1], fc2_W.shape[1]
    assert N % NC == 0, N
    NPC = N // NC
    NBLK = (NPC + P - 1) // P
    NPAD = NBLK * P
    NTAB = NC * NPAD
    assert NTAB <= 65536
    BIAS = max(0, NTAB - 32768)
    KD = D // P

    src = np.asarray(edge_index[0], np.int64)
    dst = np.asarray(edge_index[1], np.int64)
    deg = np.bincount(dst, minlength=N).astype(np.float32) + 1.0
    dis = 1.0 / np.sqrt(deg)

    # fold LN gain + BN(eval) into weights; biases:
    #   z1 = LN(x) @ W1f + zb1, post-agg bias b1f
    k1 = bn1_g / np.sqrt(bn1_v + EPS)
    W1f = (np.asarray(ln_g)[:, None] * np.asarray(W1)) * k1[None, :]
    zb1 = (np.asarray(ln_b) @ np.asarray(W1)) * k1
    b1f = np.asarray(b1) * k1 + (bn1_b - bn1_m * k1)
    w1s = W1f.sum(0)                      # colsum for the LN -mu term
    k2 = bn2_g / np.sqrt(bn2_v + EPS)
    W2f = np.asarray(W2) * k2[None, :]
    b2f = np.asarray(b2) * k2 + (bn2_b - bn2_m * k2)
    k3 = bn3_g / np.sqrt(bn3_v + EPS)
    W3f = np.asarray(W3) * k3[None, :]
    b3f = np.asarray(b3) * k3 + (bn3_b - bn3_m * k3)

    # real edges only, assigned to dst owner core, bucketed by dst block
    core_of = dst // NPC
    dloc = dst - core_of * NPC
    # z-table rows are partition-major within a core's shard:
    # node local id n (block j = n//128, lane p = n%128) -> row p*NBLK + j
    nloc = src % NPC
    srcpad = (src // NPC) * NPAD + (nloc % P) * NBLK + (nloc // P)

    counts = np.zeros((NC, NBLK), np.int64)
    per_core = []
    for c in range(NC):
        m = core_of == c
        s = srcpad[m]
        d = dloc[m]
        o = np.argsort(d, kind="stable")
        s, d = s[o], d[o]
        cell = (d >> 7).astype(np.int64)
        counts[c] = np.bincount(cell, minlength=NBLK)
        per_core.append((s, d, cell))

    T = np.maximum(1, -(-counts.max(0) // P))      # tiles per block, shared
    tile_off = np.concatenate([[0], np.cumsum(T)]).astype(np.int64)
    ntiles = int(tile_off[-1])

    idx16_list, dstrel_list = [], []
    for c in range(NC):
        s, d, cell = per_core[c]
        start = np.searchsorted(cell, np.arange(NBLK))
        pos = np.arange(len(cell)) - start[cell]
        slot = tile_off[cell] * P + pos
        idx_lin = np.zeros(ntiles * P, np.int32)          # pad -> row BIAS
        rel_lin = np.full(ntiles * P, 999.0, np.float32)  # pad -> no match
        idx_lin[slot] = (s - BIAS).astype(np.int32)
        rel_lin[slot] = (d & 127).astype(np.float32)
        assert idx_lin.min() >= -32768 and idx_lin.max() <= 32767
        idx16 = idx_lin.reshape(ntiles * 8, 16).T.astype(np.int16)
        idx16 = np.tile(idx16, (8, 1))                    # [128, ntiles*8]
        dstrel = rel_lin.reshape(ntiles, P).T             # [128, ntiles]
        idx16_list.append(np.ascontiguousarray(idx16))
        dstrel_list.append(np.ascontiguousarray(_to_bf(dstrel)))

    # group blocks so one dma_gather stays under the Q7 scratch limit
    groups = []  # list of (b0, nb) block ranges
    b0 = 0
    while b0 < NBLK:
        nb = 0
        while (b0 + nb < NBLK
               and (tile_off[b0 + nb + 1] - tile_off[b0]) * P <= GIDX):
            nb += 1
        nb = max(nb, 1)
        groups.append((b0, nb))
        b0 += nb

    # per-core node data: x in bf16, node-major + feature-major(transposed)
    xbf = np.asarray(x, np.float32).astype(BF16)
    xp_list, xt_list, disb_list = [], [], []
    for c in range(NC):
        xp = np.zeros((NPAD, D), BF16)
        xp[:NPC] = xbf[c * NPC:(c + 1) * NPC]
        xpm = xp.reshape(NBLK, P, D).transpose(1, 0, 2).reshape(P, NBLK * D)
        xp_list.append(np.ascontiguousarray(xpm))
        xt = np.zeros((P, KD * NPAD), BF16)   # [128, kc*NPAD + node]
        xf = xp.reshape(NPAD, KD, P)          # node, chunk, feat
        xt[:] = np.transpose(xf, (2, 1, 0)).reshape(P, KD * NPAD)
        xt_list.append(np.ascontiguousarray(xt))
        db = np.ones(NPAD, np.float32)
        db[:NPC] = dis[c * NPC:(c + 1) * NPC]
        disb_list.append(np.ascontiguousarray(db.reshape(NBLK, P).T))

    iota = np.tile(np.arange(P, dtype=np.float32), (P, 16))
    ident = np.eye(P, dtype=np.float32)

    consts = dict(
        w1=_to_bf(W1f), w2=_to_bf(W2f), w3=_to_bf(W3f),
        fc1w=_to_bf(np.asarray(fc1_W)), fc2w=_to_bf(np.asarray(fc2_W)),
        w1s=_rep(w1s), zb1=_rep(zb1), b1f=_rep(b1f), b2f=_rep(b2f),
        b3f=_rep(b3f), fc1b=_rep(fc1_b), lncg=_rep(lnc_g), lncb=_rep(lnc_b),
        fc2b=_rep(fc2_b), iota=_to_bf(iota), idn=_to_bf(ident),
    )
    in_maps = []
    for c in range(NC):
        m = dict(consts)
        m.update(xp=xp_list[c], xt=xt_list[c], disb=disb_list[c],
                 idx16=idx16_list[c], dstrel=dstrel_list[c])
        in_maps.append(m)

    cfg = dict(N=N, D=D, E=E, H1=H1, H2=H2, H3=H3, HC=HC, C=C, NPC=NPC,
               NBLK=NBLK, NPAD=NPAD, NTAB=NTAB, BIAS=BIAS, ntiles=ntiles,
               T=T.tolist(), tile_off=tile_off.tolist(), groups=groups)
    return cfg, in_maps


def build_nc(cfg):
    stop = cfg.get("stop", "")
    D, H1, H2, H3 = cfg["D"], cfg["H1"], cfg["H2"], cfg["H3"]
    HC, C = cfg["HC"], cfg["C"]
    NBLK, NPAD, NTAB, BIAS = cfg["NBLK"], cfg["NPAD"], cfg["NTAB"], cfg["BIAS"]
    ntiles, T, tile_off = cfg["ntiles"], cfg["T"], cfg["tile_off"]
    groups = cfg["groups"]
    KD = D // P
    nq = int(cfg.get("nq", NQ))
    shared_ag = int(cfg.get("shared_ag", 1))
    f8 = int(cfg.get("f8", 1))
    F8 = mybir.dt.float8e4
    ZDT = F8 if f8 else BF          # z-table dtype
    TSL = 256 if f8 else TS         # elements per 256B table row

    nc = bacc.Bacc("TRN2", target_bir_lowering=False, debug=False,
                   num_devices=NC, num_swdge_queues=nq)
    dt = nc.dram_tensor
    ap_xp = dt("xp", [P, NBLK * D], BF, kind="ExternalInput").ap()
    ap_xt = dt("xt", [P, KD * NPAD], BF, kind="ExternalInput").ap()
    ap_disb = dt("disb", [P, NBLK], F32, kind="ExternalInput").ap()
    ap_idx16 = dt("idx16", [P, ntiles * 8], mybir.dt.int16,
                  kind="ExternalInput").ap()
    ap_dstrel = dt("dstrel", [P, ntiles], BF, kind="ExternalInput").ap()
    ap_w1 = dt("w1", [D, H1], BF, kind="ExternalInput").ap()
    ap_w2 = dt("w2", [H1, H2], BF, kind="ExternalInput").ap()
    ap_w3 = dt("w3", [H2, H3], BF, kind="ExternalInput").ap()
    ap_fc1w = dt("fc1w", [H3, HC], BF, kind="ExternalInput").ap()
    ap_fc2w = dt("fc2w", [HC, C], BF, kind="ExternalInput").ap()
    reps = {}
    for nm, wd in [("w1s", H1), ("zb1", H1), ("b1f", H1), ("b2f", H2),
                   ("b3f", H3), ("fc1b", HC), ("lncg", HC), ("lncb", HC),
                   ("fc2b", C)]:
        reps[nm] = dt(nm, [P, wd], F32, kind="ExternalInput").ap()
    ap_iota = dt("iota", [P, 16 * P], BF, kind="ExternalInput").ap()
    ap_idn = dt("idn", [P, P], BF, kind="ExternalInput").ap()
    ap_out = dt("out", [NPAD, C], F32, kind="ExternalOutput").ap()

    HH = [H1, H2, H3]

    with tile.TileContext(nc) as tc:
        with (
            tc.tile_pool(name="const", bufs=1) as cp,
            tc.tile_pool(name="stage", bufs=1) as st,
            tc.tile_pool(name="work", bufs=3) as wk,
            tc.tile_pool(name="small", bufs=4) as sm,
            tc.tile_pool(name="psA", bufs=3, space="PSUM") as psA,
            tc.tile_pool(name="psZ", bufs=2, space="PSUM") as psZ,
            tc.tile_pool(name="psT", bufs=2, space="PSUM") as psT,
            tc.tile_pool(name="dram", bufs=1, space="DRAM") as dram,
        ):
            # ---- constants to SBUF
            def load_const(ap, shape, dtype):
                t = cp.tile(shape, dtype, tag=f"c{ap.tensor.name}",
                            name=f"c{ap.tensor.name}")
                nc.sync.dma_start(t[:], ap)
                return t

            t_w1 = cp.tile([P, KD * H1], BF, tag="w1")
            nc.sync.dma_start(t_w1[:].rearrange("p (k h) -> p k h", h=H1),
                              ap_w1.rearrange("(k p) h -> p k h", p=P))
            t_w2 = load_const(ap_w2, [H1, H2], BF)
            t_w3 = load_const(ap_w3, [H2, H3], BF)
            t_fc1w = load_const(ap_fc1w, [H3, HC], BF)
            t_fc2w = load_const(ap_fc2w, [HC, C], BF)
            t_rep = {}
            for nm in reps:
                t_rep[nm] = load_const(reps[nm], list(reps[nm].shape), F32)
            t_iota = load_const(ap_iota, [P, 16 * P], BF)
            t_idn = load_const(ap_idn, [P, P], BF)
            t_disb = load_const(ap_disb, [P, NBLK], F32)
            t_eps = cp.tile([P, 1], F32, tag="eps")
            nc.vector.memset(t_eps[:], float(EPS))
            t_idx = cp.tile([P, ntiles * 8], mybir.dt.int16, tag="idx")
            nc.sync.dma_start(t_idx[:], ap_idx16)
            t_drel = cp.tile([P, ntiles], BF, tag="drel")
            nc.sync.dma_start(t_drel[:], ap_dstrel)

            z_local = [dram.tile([NPAD, TSL], ZDT, tag=f"zloc{l}",
                                 name=f"zloc{l}") for l in range(3)]
            z_full = [dram.tile([NTAB, TSL], ZDT, tag=f"zfull{l}",
                                name=f"zfull{l}",
                                addr_space="Shared" if shared_ag else "Local")
                      for l in range(3)]
            zs_buf = [st.tile([P, NBLK * HH[l]], BF, tag=f"zs{l}",
                              name=f"zs{l}") for l in range(3)]
            zs_v = [zs_buf[l][:].rearrange("p (j h) -> p j h", h=HH[l])
                    for l in range(3)]
            ztab = st.tile([P, NBLK * TSL], ZDT, tag="ztab", name="ztab")
            ztab_v = ztab[:].rearrange("p (j s) -> p j s", s=TSL)

            def write_ztable(l, Fo):
                """zs_buf[l] (tight bf16) -> padded table row dtype -> HBM."""
                nc.vector.tensor_copy(ztab_v[:, :, 0:Fo], zs_v[l])
                nc.sync.dma_start(
                    z_local[l][:].rearrange("(p j) s -> p (j s)", p=P),
                    ztab[:])
            out_buf = st.tile([P, NBLK * C], F32, tag="outb")

            # pre-allocate all persistent staging tiles so the xin pool
            # (phase-A inputs) sits on top of the stack and frees cleanly
            ystage = st.tile([P, NBLK * H1], BF, tag="ystage", name="ystage")
            t_hst = st.tile([P, NBLK * H1], F32, tag="hst", name="hst")
            t_hb = st.tile([P, NBLK * H1], BF, tag="hb", name="hb")
            t_zn = st.tile([P, NBLK * H2], F32, tag="zn", name="zn")
            t_z4 = st.tile([P, NBLK * HC], F32, tag="z4", name="z4")
            t_r4 = st.tile([P, NBLK * HC], BF, tag="r4", name="r4")

            def bcast_node(t, w):
                """[128, NBLK] tile -> broadcast AP [128, NBLK, w] (0-stride)."""
                a = t[:]
                return bass.AP(a.tensor, a.offset, a.ap + [[0, w]])

            def bcast_feat(t, w):
                """[128, w] tile -> broadcast AP [128, NBLK, w] (0-stride blk)."""
                a = t[:]
                return bass.AP(a.tensor, a.offset,
                               [a.ap[0], [0, NBLK], a.ap[1]])

            # ============ phase A: stats + z1 = LN(x) @ W1f (folded) ========
            if stop != "Z":
              with tc.tile_pool(name="xin", bufs=1) as xin:
                t_xp = xin.tile([P, NBLK * D], BF, tag="xp")
                nc.sync.dma_start(t_xp[:], ap_xp)
                t_xt = xin.tile([P, KD * NPAD], BF, tag="xt")
                nc.sync.dma_start(t_xt[:], ap_xt)
                t1 = xin.tile([P, NBLK * H1], BF, tag="t1", name="t1")
                t2 = xin.tile([P, NBLK * H1], BF, tag="t2", name="t2")

                # stats: mu, rstd per node (x^2 accum on ACT, sums on DVE)
                ssum = sm.tile([P, NBLK], F32, tag="ssum")
                nc.vector.reduce_sum(
                    ssum[:].rearrange("p (j o) -> p j o", o=1),
                    t_xp[:].rearrange("p (j d) -> p j d", d=D),
                    axis=mybir.AxisListType.X)
                s2 = sm.tile([P, NBLK], F32, tag="s2")
                sqscr = wk.tile([P, D], F32, tag="sqscr")
                for b in range(NBLK):
                    nc.scalar.activation(
                        sqscr[:], t_xp[:, b * D:(b + 1) * D],
                        mybir.ActivationFunctionType.Square,
                        accum_out=s2[:, b:b + 1])
                mu = sm.tile([P, NBLK], F32, tag="mu")
                nc.vector.tensor_scalar_mul(mu[:], ssum[:], 1.0 / D)
                musq = sm.tile([P, NBLK], F32, tag="musq")
                nc.vector.tensor_tensor(musq[:], mu[:], mu[:],
                                        op=mybir.AluOpType.mult)
                var = sm.tile([P, NBLK], F32, tag="var")
                nc.vector.tensor_scalar_mul(var[:], s2[:], 1.0 / D)
                nc.vector.tensor_tensor(var[:], var[:], musq[:],
                                        op=mybir.AluOpType.subtract)
                std = sm.tile([P, NBLK], F32, tag="std")
                nc.scalar.activation(std[:], var[:],
                                     mybir.ActivationFunctionType.Sqrt,
                                     bias=t_eps[:], scale=1.0)
                rstd = sm.tile([P, NBLK], F32, tag="rstd")
                nc.vector.reciprocal(rstd[:], std[:])
                # a = disb*rstd ; m2 = -disb*rstd*mu
                a_sc = sm.tile([P, NBLK], F32, tag="a_sc")
                nc.vector.tensor_tensor(a_sc[:], t_disb[:], rstd[:],
                                        op=mybir.AluOpType.mult)
                m2 = sm.tile([P, NBLK], F32, tag="m2")
                nc.vector.tensor_tensor(m2[:], a_sc[:], mu[:],
                                        op=mybir.AluOpType.mult)
                nc.vector.tensor_scalar_mul(m2[:], m2[:], -1.0)

                # y = x @ W1f + zb1 per block on PE (zb1 added via DVE below)
                for b in range(NBLK):
                    zp = psZ.tile([P, H1], F32, tag="zps")
                    for kc in range(KD):
                        nc.tensor.matmul(
                            zp[:],
                            lhsT=t_xt[:, kc * NPAD + b * P:
                                      kc * NPAD + (b + 1) * P],
                            rhs=t_w1[:, kc * H1:(kc + 1) * H1],
                            start=(kc == 0), stop=(kc == KD - 1))
                    nc.vector.tensor_copy(ystage[:, b * H1:(b + 1) * H1],
                                          zp[:])
                # zs0 = a*y + m2*w1s + disb*zb1   (batched, bf16 out)
                nc.vector.tensor_tensor(
                    t1[:].rearrange("p (j h) -> p j h", h=H1),
                    bcast_node(m2, H1), bcast_feat(t_rep["w1s"], H1),
                    op=mybir.AluOpType.mult)
                nc.vector.tensor_tensor(
                    t2[:].rearrange("p (j h) -> p j h", h=H1),
                    bcast_node(t_disb, H1), bcast_feat(t_rep["zb1"], H1),
                    op=mybir.AluOpType.mult)
                nc.vector.tensor_tensor(t1[:], t1[:], t2[:],
                                        op=mybir.AluOpType.add)
                nc.vector.tensor_tensor(
                    t2[:].rearrange("p (j h) -> p j h", h=H1),
                    ystage[:].rearrange("p (j h) -> p j h", h=H1),
                    bcast_node(a_sc, H1), op=mybir.AluOpType.mult)
                nc.vector.tensor_tensor(
                    zs_v[0],
                    t2[:].rearrange("p (j h) -> p j h", h=H1),
                    t1[:].rearrange("p (j h) -> p j h", h=H1),
                    op=mybir.AluOpType.add)
                write_ztable(0, H1)

            RP = int(cfg.get("R", 1))
            _gq = [0]

            def edge_layer(l, Fh, Fo, t_wnext, postbias, mode="full",
                           reps_=(1, 1, 1, 1)):
                rep_ag, rep_g, rep_oh, rep_mm = reps_
                if cfg.get("no_cc"):
                    for c in range(NC):
                        nc.sync.dma_start(
                            z_full[l][c * NPAD:(c + 1) * NPAD, :],
                            z_local[l][:])
                else:
                    for ra in range(rep_ag):
                        zdst = z_full[l] if ra == 0 else dram.tile(
                            [NTAB, TSL], ZDT, tag=f"zfr{ra}", name=f"zfr{ra}",
                            addr_space="Shared" if shared_ag else "Local")
                        nc.gpsimd.collective_compute(
                            "AllGather", mybir.AluOpType.bypass,
                            replica_groups=[list(range(NC))],
                            ins=[z_local[l][:].opt()],
                            outs=[zdst[:].opt()],
                        )
                if mode == "ag":
                    return
                hstage = t_hst[:, 0:NBLK * Fh]
                for (b0, nb) in groups:
                    t0 = tile_off[b0]
                    t1_ = tile_off[b0 + nb]
                    gt = t1_ - t0
                    gbuf = gp.tile([P, gt * Fh], ZDT, tag="gbuf")
                    for _ in range(rep_g):
                        _gq[0] += 1
                        nc.gpsimd.dma_gather(
                            out_ap=gbuf[:].rearrange("p (n f) -> p n f", f=Fh),
                            in_ap=z_full[l][BIAS:, 0:Fh],
                            idxs_ap=t_idx[:, t0 * 8:t1_ * 8],
                            num_idxs=gt * P,
                            num_idxs_reg=gt * P,
                            elem_size=Fh,
                            elem_step=TSL,
                            single_packet=False,
                            queue_num=_gq[0] % nq,
                        )
                    sbuf = op_.tile([P, gt * P], ZDT, tag="sbufS")
                    if mode == "gather0":
                        nc.vector.tensor_copy(out_buf[:, 0:C], gbuf[:, 0:C])
                        continue
                    for _ in range(rep_oh):
                        for s0 in range(0, gt, 16):
                            s1 = min(s0 + 16, gt)
                            dr = t_drel[:, t0 + s0:t0 + s1]
                            dr_b = bass.AP(dr.tensor, dr.offset,
                                           dr.ap + [[0, P]])
                            nc.vector.tensor_tensor(
                                out=sbuf[:, s0 * P:s1 * P].rearrange(
                                    "p (t w) -> p t w", w=P),
                                in0=t_iota[:, 0:(s1 - s0) * P].rearrange(
                                    "p (t w) -> p t w", w=P),
                                in1=dr_b,
                                op=mybir.AluOpType.is_equal)
                    if mode == "gather":
                        nc.vector.tensor_copy(out_buf[:, 0:C], gbuf[:, 0:C])
                        nc.vector.tensor_copy(out_buf[:, C:2 * C],
                                              sbuf[:, 0:C])
                        continue
                    for b in range(b0, b0 + nb):
                        agg = psA.tile([P, Fh], F32, tag="agg")
                        nt = T[b]
                        base = tile_off[b]
                        for _ in range(rep_mm):
                            for t in range(nt):
                                g = base + t - t0
                                nc.tensor.matmul(
                                    agg[:],
                                    lhsT=sbuf[:, g * P:(g + 1) * P],
                                    rhs=gbuf[:, g * Fh:(g + 1) * Fh],
                                    start=(t == 0), stop=(t == nt - 1))
                        nc.vector.tensor_copy(
                            hstage[:, b * Fh:(b + 1) * Fh], agg[:])
                if mode in ("gather0", "gather"):
                    return
                # epilogue (batched): h = relu(disb*(agg + zs) + bias)
                nc.vector.tensor_tensor(
                    hstage[:].rearrange("p (j h) -> p j h", h=Fh),
                    hstage[:].rearrange("p (j h) -> p j h", h=Fh),
                    zs_v[l], op=mybir.AluOpType.add)
                nc.vector.tensor_tensor(
                    hstage[:].rearrange("p (j h) -> p j h", h=Fh),
                    hstage[:].rearrange("p (j h) -> p j h", h=Fh),
                    bcast_node(t_disb, Fh), op=mybir.AluOpType.mult)
                nc.vector.tensor_tensor(
                    hstage[:].rearrange("p (j h) -> p j h", h=Fh),
                    hstage[:].rearrange("p (j h) -> p j h", h=Fh),
                    bcast_feat(postbias, Fh), op=mybir.AluOpType.add)
                hb = t_hb[:, 0:NBLK * Fh]
                nc.scalar.activation(hb[:], hstage[:],
                                     mybir.ActivationFunctionType.Relu)
                if t_wnext is None:
                    return hb
                # z_{l+1} = disb * (h @ Wnext) per block, batched scale
                znext = t_zn[:, 0:NBLK * Fo]
                for b in range(NBLK):
                    tp = psT.tile([P, P], BF, tag="tps")
                    nc.tensor.transpose(tp[0:Fh, :],
                                        hb[:, b * Fh:(b + 1) * Fh], t_idn[:])
                    hT = wk.tile([P, P], BF, tag="hT")
                    nc.vector.tensor_copy(hT[0:Fh, :], tp[0:Fh, :])
                    zp = psZ.tile([P, Fo], F32, tag="zps")
                    nc.tensor.matmul(zp[:], lhsT=hT[0:Fh, :], rhs=t_wnext[:],
                                     start=True, stop=True)
                    nc.vector.tensor_copy(znext[:, b * Fo:(b + 1) * Fo],
                                          zp[:])
                nc.vector.tensor_tensor(
                    zs_v[l + 1],
                    znext[:].rearrange("p (j h) -> p j h", h=Fo),
                    bcast_node(t_disb, Fo), op=mybir.AluOpType.mult)
                write_ztable(l + 1, Fo)
                return None

            def classifier(h3):
                # z4 = h3 @ fc1W + fc1b ; r = relu(LN(z4)) ; out = r@fc2W+fc2b
                z4 = t_z4
                for b in range(NBLK):
                    tp = psT.tile([P, P], BF, tag="tps")
                    nc.tensor.transpose(tp[0:H3, :],
                                        h3[:, b * H3:(b + 1) * H3], t_idn[:])
                    hT = wk.tile([P, P], BF, tag="hT")
                    nc.vector.tensor_copy(hT[0:H3, :], tp[0:H3, :])
                    zp = psZ.tile([P, HC], F32, tag="zps")
                    nc.tensor.matmul(zp[:], lhsT=hT[0:H3, :], rhs=t_fc1w[:],
                                     start=True, stop=True)
                    nc.vector.tensor_copy(z4[:, b * HC:(b + 1) * HC], zp[:])
                nc.vector.tensor_tensor(
                    z4[:].rearrange("p (j h) -> p j h", h=HC),
                    z4[:].rearrange("p (j h) -> p j h", h=HC),
                    bcast_feat(t_rep["fc1b"], HC), op=mybir.AluOpType.add)
                # LN over HC
                ssum = sm.tile([P, NBLK], F32, tag="ssum4")
                nc.vector.reduce_sum(
                    ssum[:].rearrange("p (j o) -> p j o", o=1),
                    z4[:].rearrange("p (j h) -> p j h", h=HC),
                    axis=mybir.AxisListType.X)
                mu = sm.tile([P, NBLK], F32, tag="mu4")
                nc.vector.tensor_scalar_mul(mu[:], ssum[:], 1.0 / HC)
                zc = wk.tile([P, NBLK * HC], F32, tag="zc")
                nc.vector.tensor_tensor(
                    zc[:].rearrange("p (j h) -> p j h", h=HC),
                    z4[:].rearrange("p (j h) -> p j h", h=HC),
                    bcast_node(mu, HC), op=mybir.AluOpType.subtract)
                zsq = wk.tile([P, NBLK * HC], F32, tag="zsq")
                nc.vector.tensor_tensor(zsq[:], zc[:], zc[:],
                                        op=mybir.AluOpType.mult)
                var = sm.tile([P, NBLK], F32, tag="var4")
                nc.vector.reduce_sum(
                    var[:].rearrange("p (j o) -> p j o", o=1),
                    zsq[:].rearrange("p (j h) -> p j h", h=HC),
                    axis=mybir.AxisListType.X)
                nc.vector.tensor_scalar_mul(var[:], var[:], 1.0 / HC)
                std = sm.tile([P, NBLK], F32, tag="std4")
                nc.scalar.activation(std[:], var[:],
                                     mybir.ActivationFunctionType.Sqrt,
                                     bias=t_eps[:], scale=1.0)
                rstd = sm.tile([P, NBLK], F32, tag="rstd4")
                nc.vector.reciprocal(rstd[:], std[:])
                nc.vector.tensor_tensor(
                    zc[:].rearrange("p (j h) -> p j h", h=HC),
                    zc[:].rearrange("p (j h) -> p j h", h=HC),
                    bcast_node(rstd, HC), op=mybir.AluOpType.mult)
                nc.vector.tensor_tensor(
                    zc[:].rearrange("p (j h) -> p j h", h=HC),
                    zc[:].rearrange("p (j h) -> p j h", h=HC),
                    bcast_feat(t_rep["lncg"], HC), op=mybir.AluOpType.mult)
                nc.vector.tensor_tensor(
                    zc[:].rearrange("p (j h) -> p j h", h=HC),
                    zc[:].rearrange("p (j h) -> p j h", h=HC),
                    bcast_feat(t_rep["lncb"], HC), op=mybir.AluOpType.add)
                r4 = t_r4
                nc.scalar.activation(r4[:], zc[:],
                                     mybir.ActivationFunctionType.Relu)
                for b in range(NBLK):
                    tp2 = psT.tile([P, P], BF, tag="tps")
                    nc.tensor.transpose(tp2[0:HC, :],
                                        r4[:, b * HC:(b + 1) * HC], t_idn[:])
                    rT = wk.tile([P, P], BF, tag="rT")
                    nc.vector.tensor_copy(rT[0:HC, :], tp2[0:HC, :])
                    op2 = psZ.tile([P, C], F32, tag="zps")
                    nc.tensor.matmul(op2[:], lhsT=rT[0:HC, :], rhs=t_fc2w[:],
                                     start=True, stop=True)
                    nc.vector.tensor_copy(out_buf[:, b * C:(b + 1) * C],
                                          op2[:])
                nc.vector.tensor_tensor(
                    out_buf[:].rearrange("p (j c) -> p j c", c=C),
                    out_buf[:].rearrange("p (j c) -> p j c", c=C),
                    bcast_feat(t_rep["fc2b"], C), op=mybir.AluOpType.add)

            gbn = int(cfg.get("gbufs", 3))
            with (
                tc.tile_pool(name="gath", bufs=gbn) as gp,
                tc.tile_pool(name="onehot", bufs=gbn) as op_,
            ):
                if stop == "Z":
                    nc.vector.memset(out_buf[:], 0.0)
                elif stop == "A":
                    nc.vector.memset(out_buf[:], 0.0)
                elif stop in ("AG", "G0", "G1", "L1"):
                    edge_layer(0, H1, H2, t_w2, t_rep["b1f"],
                               mode={"AG": "ag", "G0": "gather0",
                                     "G1": "gather", "L1": "full"}[stop])
                    nc.vector.memset(out_buf[:], 0.0)
                elif stop in ("AGR", "G0R", "G1R", "L1R"):
                    md = {"AGR": "ag", "G0R": "gather0", "G1R": "gather",
                          "L1R": "full"}[stop]
                    rp = {"AGR": (RP, 1, 1, 1), "G0R": (1, RP, 1, 1),
                          "G1R": (1, 1, RP, 1), "L1R": (1, 1, 1, RP)}[stop]
                    edge_layer(0, H1, H2, t_w2, t_rep["b1f"], mode=md,
                               reps_=rp)
                    nc.vector.memset(out_buf[:], 0.0)
                else:
                    edge_layer(0, H1, H2, t_w2, t_rep["b1f"])
                    edge_layer(1, H2, H3, t_w3, t_rep["b2f"])
                    h3 = edge_layer(2, H3, None, None, t_rep["b3f"])
                    classifier(h3)

            nc.sync.dma_start(
                ap_out.rearrange("(j p) c -> p j c", p=P),
                out_buf[:].rearrange("p (j c) -> p j c", c=C))
    nc.compile()
    return nc


_CACHE = {}


def _get_nc(cfg):
    key = repr(sorted((k, str(v)) for k, v in cfg.items()))
    if key not in _CACHE:
        _CACHE[key] = build_nc(cfg)
    return _CACHE[key]


def kernel(**inputs):
    cfg, in_maps = preprocess(**inputs)
    nc = _get_nc(cfg)
    res = bass_utils.run_bass_kernel_spmd(nc, in_maps, core_ids=list(range(NC)))
    NPC, NPAD, N, C = cfg["NPC"], cfg["NPAD"], cfg["N"], cfg["C"]
    out = np.empty((N, C), np.float32)
    for c in range(NC):
        out[c * NPC:(c + 1) * NPC] = res.results[c]["out"][:NPC]
    return out


# revision 21
# speedup vs baseline: 4.8788x; 4.8788x over previous
"""Distributed GCN (3x GCNConv + MLP classifier) on 8 Trainium2 NeuronCores, v2.

Strategy (graph/data parallel, dst-partitioned):
  - nodes are partitioned into 8 contiguous chunks (one per core), padded to a
    multiple of 128; real edges (no self-loops) are assigned to the core
    owning their dst node and bucketed by dst block of 128; self-loop
    contributions are applied algebraically in the epilogue.
  - phase A folds LayerNorm through the first matmul via linearity:
    z = rstd*(x @ Wf) - rstd*mu*colsum(Wf) + zb1, so no elementwise normalize
    pass is needed; x is supplied in bf16 twice (node-major for stats,
    feature-major for the PE matmul).
  - per layer: AllGather the dis-scaled z-table (bf16, 256B row stride,
    Shared-output collective), dma_gather z[src] rows per 128-edge tile on two
    SWDGE queues, aggregate agg[dst] += onehot(dst_rel).T @ z_rows on the
    TensorEngine (PSUM f32), batched epilogue across all blocks.
All heavy math runs on device; the host only preprocesses integer graph
structure, folds parameters, and slices/concatenates per-core arrays.
"""
import sys

for _p in ("/opt/trn_rl_repo",):
    if _p not in sys.path:
        sys.path.insert(0, _p)

import numpy as np
import ml_dtypes

import concourse.bass as bass
import concourse.bacc as bacc
import concourse.tile as tile
import concourse.mybir as mybir
from concourse import bass_utils

BF16 = ml_dtypes.bfloat16
F32 = mybir.dt.float32
BF = mybir.dt.bfloat16
EPS = 1e-5
NC = 8
P = 128          # partitions / block size
TS = 128         # z-table row stride in bf16 elements (256B, dma_gather req)
GIDX = 8192      # max gather indices per dma_gather call
NQ = 2           # SWDGE queues for gather round-robin


def _patch_dma_gather():
    """Relax bass's elem_size%256B assert: the Q7 ucode only needs the row
    STRIDE to be a multiple of 256B; the payload can be narrower."""
    import inspect, re, textwrap
    import concourse.bass as cbass
    src = textwrap.dedent(inspect.getsource(cbass.BassGpSimd.dma_gather))
    if "elem_size_bytes > 0 and elem_size_bytes % 256 == 0" not in src:
        return  # already patched
    pat = re.compile(
        r"assert \(\s*elem_size_bytes > 0 and elem_size_bytes % 256 == 0\s*\)"
        r"\s*#[^\n]*", re.S)
    assert pat.search(src), "dma_gather source changed; update patch"
    src = pat.sub("assert elem_size_bytes > 0", src)
    ns = vars(cbass).copy()
    exec(compile(src, "<patched_dma_gather>", "exec"), ns)
    cbass.BassGpSimd.dma_gather = ns["dma_gather"]


_patch_dma_gather()


def _to_bf(a):
    return np.ascontiguousarray(np.asarray(a, np.float32)).astype(BF16)


def _rep(v):
    """Replicate a 1-D param across 128 partitions -> [128, len]."""
    v = np.asarray(v, np.float32).reshape(1, -1)
    return np.ascontiguousarray(np.repeat(v, P, 0))


def preprocess(x, edge_index, ln_g, ln_b, W1, b1, bn1_g, bn1_b, bn1_m, bn1_v,
               W2, b2, bn2_g, bn2_b, bn2_m, bn2_v, W3, b3, bn3_g, bn3_b, bn3_m,
               bn3_v, fc1_W, fc1_b, lnc_g, lnc_b, fc2_W, fc2_b):
    N, D = x.shape
    E = edge_index.shape[1]
    H1, H2, H3 = W1.shape[1], W2.shape[1], W3.shape[1]
    HC, C = fc1_W.shape[1], fc2_W.shape[1]
    assert N % NC == 0, N
    NPC = N // NC
    NBLK = (NPC + P - 1) // P
    NPAD = NBLK * P
    NTAB = NC * NPAD
    assert NTAB <= 65536
    BIAS = max(0, NTAB - 32768)
    KD = D // P

    src = np.asarray(edge_index[0], np.int64)
    dst = np.asarray(edge_index[1], np.int64)
    deg = np.bincount(dst, minlength=N).astype(np.float32) + 1.0
    dis = 1.0 / np.sqrt(deg)

    # fold LN gain + BN(eval) into weights; biases:
    #   z1 = LN(x) @ W1f + zb1, post-agg bias b1f
    k1 = bn1_g / np.sqrt(bn1_v + EPS)
    W1f = (np.asarray(ln_g)[:, None] * np.asarray(W1)) * k1[None, :]
    zb1 = (np.asarray(ln_b) @ np.asarray(W1)) * k1
    b1f = np.asarray(b1) * k1 + (bn1_b - bn1_m * k1)
    w1s = W1f.sum(0)                      # colsum for the LN -mu term
    k2 = bn2_g / np.sqrt(bn2_v + EPS)
    W2f = np.asarray(W2) * k2[None, :]
    b2f = np.asarray(b2) * k2 + (bn2_b - bn2_m * k2)
    k3 = bn3_g / np.sqrt(bn3_v + EPS)
    W3f = np.asarray(W3) * k3[None, :]
    b3f = np.asarray(b3) * k3 + (bn3_b - bn3_m * k3)

    # real edges only, assigned to dst owner core, bucketed by dst block
    core_of = dst // NPC
    dloc = dst - core_of * NPC
    # z-table rows are partition-major within a core's shard:
    # node local id n (block j = n//128, lane p = n%128) -> row p*NBLK + j
    nloc = src % NPC
    srcpad = (src // NPC) * NPAD + (nloc % P) * NBLK + (nloc // P)

    counts = np.zeros((NC, NBLK), np.int64)
    per_core = []
    for c in range(NC):
        m = core_of == c
        s = srcpad[m]
        d = dloc[m]
        o = np.argsort(d, kind="stable")
        s, d = s[o], d[o]
        cell = (d >> 7).astype(np.int64)
        counts[c] = np.bincount(cell, minlength=NBLK)
        per_core.append((s, d, cell))

    T = np.maximum(1, -(-counts.max(0) // P))      # tiles per block, shared
    tile_off = np.concatenate([[0], np.cumsum(T)]).astype(np.int64)
    ntiles = int(tile_off[-1])

    idx16_list, dstrel_list = [], []
    for c in range(NC):
        s, d, cell = per_core[c]
        start = np.searchsorted(cell, np.arange(NBLK))
        pos = np.arange(len(cell)) - start[cell]
        slot = tile_off[cell] * P + pos
        idx_lin = np.zeros(ntiles * P, np.int32)          # pad -> row BIAS
        rel_lin = np.full(ntiles * P, 999.0, np.float32)  # pad -> no match
        idx_lin[slot] = (s - BIAS).astype(np.int32)
        rel_lin[slot] = (d & 127).astype(np.float32)
        assert idx_lin.min() >= -32768 and idx_lin.max() <= 32767
        idx16 = idx_lin.reshape(ntiles * 8, 16).T.astype(np.int16)
        idx16 = np.tile(idx16, (8, 1))                    # [128, ntiles*8]
        dstrel = rel_lin.reshape(ntiles, P).T             # [128, ntiles]
        idx16_list.append(np.ascontiguousarray(idx16))
        dstrel_list.append(np.ascontiguousarray(_to_bf(dstrel)))

    # group blocks so one dma_gather stays under the Q7 scratch limit
    groups = []  # list of (b0, nb) block ranges
    b0 = 0
    while b0 < NBLK:
        nb = 0
        while (b0 + nb < NBLK
               and (tile_off[b0 + nb + 1] - tile_off[b0]) * P <= GIDX):
            nb += 1
        nb = max(nb, 1)
        groups.append((b0, nb))
        b0 += nb

    # per-core node data: x in bf16, node-major + feature-major(transposed)
    xbf = np.asarray(x, np.float32).astype(BF16)
    xp_list, xt_list, disb_list = [], [], []
    for c in range(NC):
        xp = np.zeros((NPAD, D), BF16)
        xp[:NPC] = xbf[c * NPC:(c + 1) * NPC]
        xpm = xp.reshape(NBLK, P, D).transpose(1, 0, 2).reshape(P, NBLK * D)
        xp_list.append(np.ascontiguousarray(xpm))
        xt = np.zeros((P, KD * NPAD), BF16)   # [128, kc*NPAD + node]
        xf = xp.reshape(NPAD, KD, P)          # node, chunk, feat
        xt[:] = np.transpose(xf, (2, 1, 0)).reshape(P, KD * NPAD)
        xt_list.append(np.ascontiguousarray(xt))
        db = np.ones(NPAD, np.float32)
        db[:NPC] = dis[c * NPC:(c + 1) * NPC]
        disb_list.append(np.ascontiguousarray(db.reshape(NBLK, P).T))

    iota = np.tile(np.arange(P, dtype=np.float32), (P, 16))
    ident = np.eye(P, dtype=np.float32)

    consts = dict(
        w1=_to_bf(W1f), w2=_to_bf(W2f), w3=_to_bf(W3f),
        fc1w=_to_bf(np.asarray(fc1_W)), fc2w=_to_bf(np.asarray(fc2_W)),
        w1s=_rep(w1s), zb1=_rep(zb1), b1f=_rep(b1f), b2f=_rep(b2f),
        b3f=_rep(b3f), fc1b=_rep(fc1_b), lncg=_rep(lnc_g), lncb=_rep(lnc_b),
        fc2b=_rep(fc2_b), iota=_to_bf(iota), idn=_to_bf(ident),
    )
    in_maps = []
    for c in range(NC):
        m = dict(consts)
        m.update(xp=xp_list[c], xt=xt_list[c], disb=disb_list[c],
                 idx16=idx16_list[c], dstrel=dstrel_list[c])
        in_maps.append(m)

    cfg = dict(N=N, D=D, E=E, H1=H1, H2=H2, H3=H3, HC=HC, C=C, NPC=NPC,
               NBLK=NBLK, NPAD=NPAD, NTAB=NTAB, BIAS=BIAS, ntiles=ntiles,
               T=T.tolist(), tile_off=tile_off.tolist(), groups=groups)
    return cfg, in_maps


def build_nc(cfg):
    stop = cfg.get("stop", "")
    D, H1, H2, H3 = cfg["D"], cfg["H1"], cfg["H2"], cfg["H3"]
    HC, C = cfg["HC"], cfg["C"]
    NBLK, NPAD, NTAB, BIAS = cfg["NBLK"], cfg["NPAD"], cfg["NTAB"], cfg["BIAS"]
    ntiles, T, tile_off = cfg["ntiles"], cfg["T"], cfg["tile_off"]
    groups = cfg["groups"]
    KD = D // P
    nq = int(cfg.get("nq", NQ))
    shared_ag = int(cfg.get("shared_ag", 1))
    f8 = int(cfg.get("f8", 1))
    F8 = mybir.dt.float8e4
    ZDT = F8 if f8 else BF          # z-table dtype
    TSL = 256 if f8 else TS         # elements per 256B table row

    nc = bacc.Bacc("TRN2", target_bir_lowering=False, debug=False,
                   num_devices=NC, num_swdge_queues=nq)
    dt = nc.dram_tensor
    ap_xp = dt("xp", [P, NBLK * D], BF, kind="ExternalInput").ap()
    ap_xt = dt("xt", [P, KD * NPAD], BF, kind="ExternalInput").ap()
    ap_disb = dt("disb", [P, NBLK], F32, kind="ExternalInput").ap()
    ap_idx16 = dt("idx16", [P, ntiles * 8], mybir.dt.int16,
                  kind="ExternalInput").ap()
    ap_dstrel = dt("dstrel", [P, ntiles], BF, kind="ExternalInput").ap()
    ap_w1 = dt("w1", [D, H1], BF, kind="ExternalInput").ap()
    ap_w2 = dt("w2", [H1, H2], BF, kind="ExternalInput").ap()
    ap_w3 = dt("w3", [H2, H3], BF, kind="ExternalInput").ap()
    ap_fc1w = dt("fc1w", [H3, HC], BF, kind="ExternalInput").ap()
    ap_fc2w = dt("fc2w", [HC, C], BF, kind="ExternalInput").ap()
    reps = {}
    for nm, wd in [("w1s", H1), ("zb1", H1), ("b1f", H1), ("b2f", H2),
                   ("b3f", H3), ("fc1b", HC), ("lncg", HC), ("lncb", HC),
                   ("fc2b", C)]:
        reps[nm] = dt(nm, [P, wd], F32, kind="ExternalInput").ap()
    ap_iota = dt("iota", [P, 16 * P], BF, kind="ExternalInput").ap()
    ap_idn = dt("idn", [P, P], BF, kind="ExternalInput").ap()
    ap_out = dt("out", [NPAD, C], F32, kind="ExternalOutput").ap()

    HH = [H1, H2, H3]

    with tile.TileContext(nc) as tc:
        with (
            tc.tile_pool(name="const", bufs=1) as cp,
            tc.tile_pool(name="stage", bufs=1) as st,
            tc.tile_pool(name="work", bufs=3) as wk,
            tc.tile_pool(name="small", bufs=4) as sm,
            tc.tile_pool(name="psA", bufs=3, space="PSUM") as psA,
            tc.tile_pool(name="psZ", bufs=2, space="PSUM") as psZ,
            tc.tile_pool(name="psT", bufs=2, space="PSUM") as psT,
            tc.tile_pool(name="dram", bufs=1, space="DRAM") as dram,
        ):
            # ---- constants to SBUF
            def load_const(ap, shape, dtype):
                t = cp.tile(shape, dtype, tag=f"c{ap.tensor.name}",
                            name=f"c{ap.tensor.name}")
                nc.sync.dma_start(t[:], ap)
                return t

            t_w1 = cp.tile([P, KD * H1], BF, tag="w1")
            nc.sync.dma_start(t_w1[:].rearrange("p (k h) -> p k h", h=H1),
                              ap_w1.rearrange("(k p) h -> p k h", p=P))
            t_w2 = load_const(ap_w2, [H1, H2], BF)
            t_w3 = load_const(ap_w3, [H2, H3], BF)
            t_fc1w = load_const(ap_fc1w, [H3, HC], BF)
            t_fc2w = load_const(ap_fc2w, [HC, C], BF)
            t_rep = {}
            for nm in reps:
                t_rep[nm] = load_const(reps[nm], list(reps[nm].shape), F32)
            t_iota = load_const(ap_iota, [P, 16 * P], BF)
            t_idn = load_const(ap_idn, [P, P], BF)
            t_disb = load_const(ap_disb, [P, NBLK], F32)
            t_eps = cp.tile([P, 1], F32, tag="eps")
            nc.vector.memset(t_eps[:], float(EPS))
            t_idx = cp.tile([P, ntiles * 8], mybir.dt.int16, tag="idx")
            nc.sync.dma_start(t_idx[:], ap_idx16)
            t_drel = cp.tile([P, ntiles], BF, tag="drel")
            nc.sync.dma_start(t_drel[:], ap_dstrel)

            z_local = [dram.tile([NPAD, TSL], ZDT, tag=f"zloc{l}",
                                 name=f"zloc{l}") for l in range(3)]
            z_full = [dram.tile([NTAB, TSL], ZDT, tag=f"zfull{l}",
                                name=f"zfull{l}",
                                addr_space="Shared" if shared_ag else "Local")
                      for l in range(3)]
            zs_buf = [st.tile([P, NBLK * HH[l]], BF, tag=f"zs{l}",
                              name=f"zs{l}") for l in range(3)]
            zs_v = [zs_buf[l][:].rearrange("p (j h) -> p j h", h=HH[l])
                    for l in range(3)]
            ztab = st.tile([P, NBLK * TSL], ZDT, tag="ztab", name="ztab")
            ztab_v = ztab[:].rearrange("p (j s) -> p j s", s=TSL)

            def write_ztable(l, Fo):
                """zs_buf[l] (tight bf16) -> padded table row dtype -> HBM."""
                nc.vector.tensor_copy(ztab_v[:, :, 0:Fo], zs_v[l])
                nc.sync.dma_start(
                    z_local[l][:].rearrange("(p j) s -> p (j s)", p=P),
                    ztab[:])
            out_buf = st.tile([P, NBLK * C], F32, tag="outb")

            # pre-allocate all persistent staging tiles so the xin pool
            # (phase-A inputs) sits on top of the stack and frees cleanly
            ystage = st.tile([P, NBLK * H1], BF, tag="ystage", name="ystage")
            t_hst = st.tile([P, NBLK * H1], F32, tag="hst", name="hst")
            t_hb = st.tile([P, NBLK * H1], BF, tag="hb", name="hb")
            t_zn = st.tile([P, NBLK * H2], F32, tag="zn", name="zn")
            t_z4 = st.tile([P, NBLK * HC], F32, tag="z4", name="z4")
            t_r4 = st.tile([P, NBLK * HC], BF, tag="r4", name="r4")

            def bcast_node(t, w):
                """[128, NBLK] tile -> broadcast AP [128, NBLK, w] (0-stride)."""
                a = t[:]
                return bass.AP(a.tensor, a.offset, a.ap + [[0, w]])

            def bcast_feat(t, w):
                """[128, w] tile -> broadcast AP [128, NBLK, w] (0-stride blk)."""
                a = t[:]
                return bass.AP(a.tensor, a.offset,
                               [a.ap[0], [0, NBLK], a.ap[1]])

            # ============ phase A: stats + z1 = LN(x) @ W1f (folded) ========
            if stop != "Z":
              with tc.tile_pool(name="xin", bufs=1) as xin:
                t_xp = xin.tile([P, NBLK * D], BF, tag="xp")
                nc.sync.dma_start(t_xp[:], ap_xp)
                t_xt = xin.tile([P, KD * NPAD], BF, tag="xt")
                nc.sync.dma_start(t_xt[:], ap_xt)
                t1 = xin.tile([P, NBLK * H1], BF, tag="t1", name="t1")
                t2 = xin.tile([P, NBLK * H1], BF, tag="t2", name="t2")

                # stats: mu, rstd per node (x^2 accum on ACT, sums on DVE)
                ssum = sm.tile([P, NBLK], F32, tag="ssum")
                nc.vector.reduce_sum(
                    ssum[:].rearrange("p (j o) -> p j o", o=1),
                    t_xp[:].rearrange("p (j d) -> p j d", d=D),
                    axis=mybir.AxisListType.X)
                s2 = sm.tile([P, NBLK], F32, tag="s2")
                sqscr = wk.tile([P, D], F32, tag="sqscr")
                for b in range(NBLK):
                    nc.scalar.activation(
                        sqscr[:], t_xp[:, b * D:(b + 1) * D],
                        mybir.ActivationFunctionType.Square,
                        accum_out=s2[:, b:b + 1])
                mu = sm.tile([P, NBLK], F32, tag="mu")
                nc.vector.tensor_scalar_mul(mu[:], ssum[:], 1.0 / D)
                musq = sm.tile([P, NBLK], F32, tag="musq")
                nc.vector.tensor_tensor(musq[:], mu[:], mu[:],
                                        op=mybir.AluOpType.mult)
                var = sm.tile([P, NBLK], F32, tag="var")
                nc.vector.tensor_scalar_mul(var[:], s2[:], 1.0 / D)
                nc.vector.tensor_tensor(var[:], var[:], musq[:],
                                        op=mybir.AluOpType.subtract)
                std = sm.tile([P, NBLK], F32, tag="std")
                nc.scalar.activation(std[:], var[:],
                                     mybir.ActivationFunctionType.Sqrt,
                                     bias=t_eps[:], scale=1.0)
                rstd = sm.tile([P, NBLK], F32, tag="rstd")
                nc.vector.reciprocal(rstd[:], std[:])
                # a = disb*rstd ; m2 = -disb*rstd*mu
                a_sc = sm.tile([P, NBLK], F32, tag="a_sc")
                nc.vector.tensor_tensor(a_sc[:], t_disb[:], rstd[:],
                                        op=mybir.AluOpType.mult)
                m2 = sm.tile([P, NBLK], F32, tag="m2")
                nc.vector.tensor_tensor(m2[:], a_sc[:], mu[:],
                                        op=mybir.AluOpType.mult)
                nc.vector.tensor_scalar_mul(m2[:], m2[:], -1.0)

                # y = x @ W1f + zb1 per block on PE (zb1 added via DVE below)
                for b in range(NBLK):
                    zp = psZ.tile([P, H1], F32, tag="zps")
                    for kc in range(KD):
                        nc.tensor.matmul(
                            zp[:],
                            lhsT=t_xt[:, kc * NPAD + b * P:
                                      kc * NPAD + (b + 1) * P],
                            rhs=t_w1[:, kc * H1:(kc + 1) * H1],
                            start=(kc == 0), stop=(kc == KD - 1))
                    nc.vector.tensor_copy(ystage[:, b * H1:(b + 1) * H1],
                                          zp[:])
                # zs0 = a*y + m2*w1s + disb*zb1   (batched, bf16 out)
                nc.vector.tensor_tensor(
                    t1[:].rearrange("p (j h) -> p j h", h=H1),
                    bcast_node(m2, H1), bcast_feat(t_rep["w1s"], H1),
                    op=mybir.AluOpType.mult)
                nc.vector.tensor_tensor(
                    t2[:].rearrange("p (j h) -> p j h", h=H1),
                    bcast_node(t_disb, H1), bcast_feat(t_rep["zb1"], H1),
                    op=mybir.AluOpType.mult)
                nc.vector.tensor_tensor(t1[:], t1[:], t2[:],
                                        op=mybir.AluOpType.add)
                nc.vector.tensor_tensor(
                    t2[:].rearrange("p (j h) -> p j h", h=H1),
                    ystage[:].rearrange("p (j h) -> p j h", h=H1),
                    bcast_node(a_sc, H1), op=mybir.AluOpType.mult)
                nc.vector.tensor_tensor(
                    zs_v[0],
                    t2[:].rearrange("p (j h) -> p j h", h=H1),
                    t1[:].rearrange("p (j h) -> p j h", h=H1),
                    op=mybir.AluOpType.add)
                write_ztable(0, H1)

            RP = int(cfg.get("R", 1))
            _gq = [0]

            def edge_layer(l, Fh, Fo, t_wnext, postbias, mode="full",
                           reps_=(1, 1, 1, 1), zf=None):
                rep_ag, rep_g, rep_oh, rep_mm = reps_
                if zf is None:
                    zf = z_full
                if cfg.get("no_cc"):
                    for c in range(NC):
                        nc.sync.dma_start(
                            zf[l][c * NPAD:(c + 1) * NPAD, :],
                            z_local[l][:])
                else:
                    for ra in range(rep_ag):
                        zdst = zf[l] if ra == 0 else dram.tile(
                            [NTAB, TSL], ZDT, tag=f"zfr{ra}", name=f"zfr{ra}",
                            addr_space="Shared" if shared_ag else "Local")
                        nc.gpsimd.collective_compute(
                            "AllGather", mybir.AluOpType.bypass,
                            replica_groups=[list(range(NC))],
                            ins=[z_local[l][:].opt()],
                            outs=[zdst[:].opt()],
                        )
                if mode == "ag":
                    return
                hstage = t_hst[:, 0:NBLK * Fh]
                for (b0, nb) in groups:
                    t0 = tile_off[b0]
                    t1_ = tile_off[b0 + nb]
                    gt = t1_ - t0
                    gbuf = gp.tile([P, gt * Fh], ZDT, tag="gbuf")
                    for _ in range(rep_g):
                        _gq[0] += 1
                        nc.gpsimd.dma_gather(
                            out_ap=gbuf[:].rearrange("p (n f) -> p n f", f=Fh),
                            in_ap=zf[l][BIAS:, 0:Fh],
                            idxs_ap=t_idx[:, t0 * 8:t1_ * 8],
                            num_idxs=gt * P,
                            num_idxs_reg=gt * P,
                            elem_size=Fh,
                            elem_step=TSL,
                            single_packet=False,
                            queue_num=_gq[0] % nq,
                        )
                    sbuf = op_.tile([P, gt * P], ZDT, tag="sbufS")
                    if mode == "gather0":
                        nc.vector.tensor_copy(out_buf[:, 0:C], gbuf[:, 0:C])
                        continue
                    for _ in range(rep_oh):
                        for s0 in range(0, gt, 16):
                            s1 = min(s0 + 16, gt)
                            dr = t_drel[:, t0 + s0:t0 + s1]
                            dr_b = bass.AP(dr.tensor, dr.offset,
                                           dr.ap + [[0, P]])
                            nc.vector.tensor_tensor(
                                out=sbuf[:, s0 * P:s1 * P].rearrange(
                                    "p (t w) -> p t w", w=P),
                                in0=t_iota[:, 0:(s1 - s0) * P].rearrange(
                                    "p (t w) -> p t w", w=P),
                                in1=dr_b,
                                op=mybir.AluOpType.is_equal)
                    if mode == "gather":
                        nc.vector.tensor_copy(out_buf[:, 0:C], gbuf[:, 0:C])
                        nc.vector.tensor_copy(out_buf[:, C:2 * C],
                                              sbuf[:, 0:C])
                        continue
                    for b in range(b0, b0 + nb):
                        agg = psA.tile([P, Fh], F32, tag="agg")
                        nt = T[b]
                        base = tile_off[b]
                        for _ in range(rep_mm):
                            for t in range(nt):
                                g = base + t - t0
                                nc.tensor.matmul(
                                    agg[:],
                                    lhsT=sbuf[:, g * P:(g + 1) * P],
                                    rhs=gbuf[:, g * Fh:(g + 1) * Fh],
                                    start=(t == 0), stop=(t == nt - 1))
                        nc.vector.tensor_copy(
                            hstage[:, b * Fh:(b + 1) * Fh], agg[:])
                if mode in ("gather0", "gather"):
                    return
                # epilogue (batched): h = relu(disb*(agg + zs) + bias)
                nc.vector.tensor_tensor(
                    hstage[:].rearrange("p (j h) -> p j h", h=Fh),
                    hstage[:].rearrange("p (j h) -> p j h", h=Fh),
                    zs_v[l], op=mybir.AluOpType.add)
                nc.vector.tensor_tensor(
                    hstage[:].rearrange("p (j h) -> p j h", h=Fh),
                    hstage[:].rearrange("p (j h) -> p j h", h=Fh),
                    bcast_node(t_disb, Fh), op=mybir.AluOpType.mult)
                nc.vector.tensor_tensor(
                    hstage[:].rearrange("p (j h) -> p j h", h=Fh),
                    hstage[:].rearrange("p (j h) -> p j h", h=Fh),
                    bcast_feat(postbias, Fh), op=mybir.AluOpType.add)
                hb = t_hb[:, 0:NBLK * Fh]
                nc.scalar.activation(hb[:], hstage[:],
                                     mybir.ActivationFunctionType.Relu)
                if t_wnext is None:
                    return hb
                # z_{l+1} = disb * (h @ Wnext) per block, batched scale
                znext = t_zn[:, 0:NBLK * Fo]
                for b in range(NBLK):
                    tp = psT.tile([P, P], BF, tag="tps")
                    nc.tensor.transpose(tp[0:Fh, :],
                                        hb[:, b * Fh:(b + 1) * Fh], t_idn[:])
                    hT = wk.tile([P, P], BF, tag="hT")
                    nc.vector.tensor_copy(hT[0:Fh, :], tp[0:Fh, :])
                    zp = psZ.tile([P, Fo], F32, tag="zps")
                    nc.tensor.matmul(zp[:], lhsT=hT[0:Fh, :], rhs=t_wnext[:],
                                     start=True, stop=True)
                    nc.vector.tensor_copy(znext[:, b * Fo:(b + 1) * Fo],
                                          zp[:])
                nc.vector.tensor_tensor(
                    zs_v[l + 1],
                    znext[:].rearrange("p (j h) -> p j h", h=Fo),
                    bcast_node(t_disb, Fo), op=mybir.AluOpType.mult)
                write_ztable(l + 1, Fo)
                return None

            def classifier(h3):
                # z4 = h3 @ fc1W + fc1b ; r = relu(LN(z4)) ; out = r@fc2W+fc2b
                z4 = t_z4
                for b in range(NBLK):
                    tp = psT.tile([P, P], BF, tag="tps")
                    nc.tensor.transpose(tp[0:H3, :],
                                        h3[:, b * H3:(b + 1) * H3], t_idn[:])
                    hT = wk.tile([P, P], BF, tag="hT")
                    nc.vector.tensor_copy(hT[0:H3, :], tp[0:H3, :])
                    zp = psZ.tile([P, HC], F32, tag="zps")
                    nc.tensor.matmul(zp[:], lhsT=hT[0:H3, :], rhs=t_fc1w[:],
                                     start=True, stop=True)
                    nc.vector.tensor_copy(z4[:, b * HC:(b + 1) * HC], zp[:])
                nc.vector.tensor_tensor(
                    z4[:].rearrange("p (j h) -> p j h", h=HC),
                    z4[:].rearrange("p (j h) -> p j h", h=HC),
                    bcast_feat(t_rep["fc1b"], HC), op=mybir.AluOpType.add)
                # LN over HC
                ssum = sm.tile([P, NBLK], F32, tag="ssum4")
                nc.vector.reduce_sum(
                    ssum[:].rearrange("p (j o) -> p j o", o=1),
                    z4[:].rearrange("p (j h) -> p j h", h=HC),
                    axis=mybir.AxisListType.X)
                mu = sm.tile([P, NBLK], F32, tag="mu4")
                nc.vector.tensor_scalar_mul(mu[:], ssum[:], 1.0 / HC)
                zc = wk.tile([P, NBLK * HC], F32, tag="zc")
                nc.vector.tensor_tensor(
                    zc[:].rearrange("p (j h) -> p j h", h=HC),
                    z4[:].rearrange("p (j h) -> p j h", h=HC),
                    bcast_node(mu, HC), op=mybir.AluOpType.subtract)
                zsq = wk.tile([P, NBLK * HC], F32, tag="zsq")
                nc.vector.tensor_tensor(zsq[:], zc[:], zc[:],
                                        op=mybir.AluOpType.mult)
                var = sm.tile([P, NBLK], F32, tag="var4")
                nc.vector.reduce_sum(
                    var[:].rearrange("p (j o) -> p j o", o=1),
                    zsq[:].rearrange("p (j h) -> p j h", h=HC),
                    axis=mybir.AxisListType.X)
                nc.vector.tensor_scalar_mul(var[:], var[:], 1.0 / HC)
                std = sm.tile([P, NBLK], F32, tag="std4")
                nc.scalar.activation(std[:], var[:],
                                     mybir.ActivationFunctionType.Sqrt,
                                     bias=t_eps[:], scale=1.0)
                rstd = sm.tile([P, NBLK], F32, tag="rstd4")
                nc.vector.reciprocal(rstd[:], std[:])
                nc.vector.tensor_tensor(
                    zc[:].rearrange("p (j h) -> p j h", h=HC),
                    zc[:].rearrange("p (j h) -> p j h", h=HC),
                    bcast_node(rstd, HC), op=mybir.AluOpType.mult)
                nc.vector.tensor_tensor(
                    zc[:].rearrange("p (j h) -> p j h", h=HC),
                    zc[:].rearrange("p (j h) -> p j h", h=HC),
                    bcast_feat(t_rep["lncg"], HC), op=mybir.AluOpType.mult)
                nc.vector.tensor_tensor(
                    zc[:].rearrange("p (j h) -> p j h", h=HC),
                    zc[:].rearrange("p (j h) -> p j h", h=HC),
                    bcast_feat(t_rep["lncb"], HC), op=mybir.AluOpType.add)
                r4 = t_r4
                nc.scalar.activation(r4[:], zc[:],
                                     mybir.ActivationFunctionType.Relu)
                for b in range(NBLK):
                    tp2 = psT.tile([P, P], BF, tag="tps")
                    nc.tensor.transpose(tp2[0:HC, :],
                                        r4[:, b * HC:(b + 1) * HC], t_idn[:])
                    rT = wk.tile([P, P], BF, tag="rT")
                    nc.vector.tensor_copy(rT[0:HC, :], tp2[0:HC, :])
                    op2 = psZ.tile([P, C], F32, tag="zps")
                    nc.tensor.matmul(op2[:], lhsT=rT[0:HC, :], rhs=t_fc2w[:],
                                     start=True, stop=True)
                    nc.vector.tensor_copy(out_buf[:, b * C:(b + 1) * C],
                                          op2[:])
                nc.vector.tensor_tensor(
                    out_buf[:].rearrange("p (j c) -> p j c", c=C),
                    out_buf[:].rearrange("p (j c) -> p j c", c=C),
                    bcast_feat(t_rep["fc2b"], C), op=mybir.AluOpType.add)

            gbn = int(cfg.get("gbufs", 3))
            with (
                tc.tile_pool(name="gath", bufs=gbn) as gp,
                tc.tile_pool(name="onehot", bufs=gbn) as op_,
            ):
                if stop == "Z":
                    nc.vector.memset(out_buf[:], 0.0)
                elif stop == "A":
                    nc.vector.memset(out_buf[:], 0.0)
                elif stop in ("AG", "G0", "G1", "L1"):
                    edge_layer(0, H1, H2, t_w2, t_rep["b1f"],
                               mode={"AG": "ag", "G0": "gather0",
                                     "G1": "gather", "L1": "full"}[stop])
                    nc.vector.memset(out_buf[:], 0.0)
                elif stop in ("AGR", "G0R", "G1R", "L1R"):
                    md = {"AGR": "ag", "G0R": "gather0", "G1R": "gather",
                          "L1R": "full"}[stop]
                    rp = {"AGR": (RP, 1, 1, 1), "G0R": (1, RP, 1, 1),
                          "G1R": (1, 1, RP, 1), "L1R": (1, 1, 1, RP)}[stop]
                    edge_layer(0, H1, H2, t_w2, t_rep["b1f"], mode=md,
                               reps_=rp)
                    nc.vector.memset(out_buf[:], 0.0)
                elif stop == "FR":
                    for r in range(RP):
                        zfr = z_full if r == 0 else [
                            dram.tile([NTAB, TSL], ZDT, tag=f"zfl{r}{l}",
                                      name=f"zfl{r}{l}",
                                      addr_space=("Shared" if shared_ag
                                                  else "Local"))
                            for l in range(3)]
                        edge_layer(0, H1, H2, t_w2, t_rep["b1f"], zf=zfr)
                        edge_layer(1, H2, H3, t_w3, t_rep["b2f"], zf=zfr)
                        h3 = edge_layer(2, H3, None, None, t_rep["b3f"],
                                        zf=zfr)
                        classifier(h3)
                else:
                    edge_layer(0, H1, H2, t_w2, t_rep["b1f"])
                    edge_layer(1, H2, H3, t_w3, t_rep["b2f"])
                    h3 = edge_layer(2, H3, None, None, t_rep["b3f"])
                    classifier(h3)

            nc.sync.dma_start(
                ap_out.rearrange("(j p) c -> p j c", p=P),
                out_buf[:].rearrange("p (j c) -> p j c", c=C))
    nc.compile()
    return nc


_CACHE = {}


def _get_nc(cfg):
    key = repr(sorted((k, str(v)) for k, v in cfg.items()))
    if key not in _CACHE:
        _CACHE[key] = build_nc(cfg)
    return _CACHE[key]


def kernel(**inputs):
    cfg, in_maps = preprocess(**inputs)
    nc = _get_nc(cfg)
    res = bass_utils.run_bass_kernel_spmd(nc, in_maps, core_ids=list(range(NC)))
    NPC, NPAD, N, C = cfg["NPC"], cfg["NPAD"], cfg["N"], cfg["C"]
    out = np.empty((N, C), np.float32)
    for c in range(NC):
        out[c * NPC:(c + 1) * NPC] = res.results[c]["out"][:NPC]
    return out


# revision 22
# speedup vs baseline: 6.2370x; 1.2784x over previous
"""Distributed GCN (3x GCNConv + MLP classifier) on 8 Trainium2 NeuronCores, v2.

Strategy (graph/data parallel, dst-partitioned):
  - nodes are partitioned into 8 contiguous chunks (one per core), padded to a
    multiple of 128; real edges (no self-loops) are assigned to the core
    owning their dst node and bucketed by dst block of 128; self-loop
    contributions are applied algebraically in the epilogue.
  - phase A folds LayerNorm through the first matmul via linearity:
    z = rstd*(x @ Wf) - rstd*mu*colsum(Wf) + zb1, so no elementwise normalize
    pass is needed; x is supplied in bf16 twice (node-major for stats,
    feature-major for the PE matmul).
  - per layer: AllGather the dis-scaled z-table (bf16, 256B row stride,
    Shared-output collective), dma_gather z[src] rows per 128-edge tile on two
    SWDGE queues, aggregate agg[dst] += onehot(dst_rel).T @ z_rows on the
    TensorEngine (PSUM f32), batched epilogue across all blocks.
All heavy math runs on device; the host only preprocesses integer graph
structure, folds parameters, and slices/concatenates per-core arrays.
"""
import sys

for _p in ("/opt/trn_rl_repo",):
    if _p not in sys.path:
        sys.path.insert(0, _p)

import numpy as np
import ml_dtypes

import concourse.bass as bass
import concourse.bacc as bacc
import concourse.tile as tile
import concourse.mybir as mybir
from concourse import bass_utils

BF16 = ml_dtypes.bfloat16
F32 = mybir.dt.float32
BF = mybir.dt.bfloat16
EPS = 1e-5
NC = 8
P = 128          # partitions / block size
TS = 128         # z-table row stride in bf16 elements (256B, dma_gather req)
GIDX = 8192      # max gather indices per dma_gather call
NQ = 4           # SWDGE queues for gather round-robin


def _patch_dma_gather():
    """Relax bass's elem_size%256B assert: the Q7 ucode only needs the row
    STRIDE to be a multiple of 256B; the payload can be narrower."""
    import inspect, re, textwrap
    import concourse.bass as cbass
    src = textwrap.dedent(inspect.getsource(cbass.BassGpSimd.dma_gather))
    if "elem_size_bytes > 0 and elem_size_bytes % 256 == 0" not in src:
        return  # already patched
    pat = re.compile(
        r"assert \(\s*elem_size_bytes > 0 and elem_size_bytes % 256 == 0\s*\)"
        r"\s*#[^\n]*", re.S)
    assert pat.search(src), "dma_gather source changed; update patch"
    src = pat.sub("assert elem_size_bytes > 0", src)
    ns = vars(cbass).copy()
    exec(compile(src, "<patched_dma_gather>", "exec"), ns)
    cbass.BassGpSimd.dma_gather = ns["dma_gather"]


_patch_dma_gather()


def _to_bf(a):
    return np.ascontiguousarray(np.asarray(a, np.float32)).astype(BF16)


def _rep(v):
    """Replicate a 1-D param across 128 partitions -> [128, len]."""
    v = np.asarray(v, np.float32).reshape(1, -1)
    return np.ascontiguousarray(np.repeat(v, P, 0))


def preprocess(x, edge_index, ln_g, ln_b, W1, b1, bn1_g, bn1_b, bn1_m, bn1_v,
               W2, b2, bn2_g, bn2_b, bn2_m, bn2_v, W3, b3, bn3_g, bn3_b, bn3_m,
               bn3_v, fc1_W, fc1_b, lnc_g, lnc_b, fc2_W, fc2_b):
    N, D = x.shape
    E = edge_index.shape[1]
    H1, H2, H3 = W1.shape[1], W2.shape[1], W3.shape[1]
    HC, C = fc1_W.shape[1], fc2_W.shape[1]
    assert N % NC == 0, N
    NPC = N // NC
    NBLK = (NPC + P - 1) // P
    NPAD = NBLK * P
    NTAB = NC * NPAD
    assert NTAB <= 65536
    BIAS = max(0, NTAB - 32768)
    KD = D // P

    src = np.asarray(edge_index[0], np.int64)
    dst = np.asarray(edge_index[1], np.int64)
    deg = np.bincount(dst, minlength=N).astype(np.float32) + 1.0
    dis = 1.0 / np.sqrt(deg)

    # fold LN gain + BN(eval) into weights; biases:
    #   z1 = LN(x) @ W1f + zb1, post-agg bias b1f
    k1 = bn1_g / np.sqrt(bn1_v + EPS)
    W1f = (np.asarray(ln_g)[:, None] * np.asarray(W1)) * k1[None, :]
    zb1 = (np.asarray(ln_b) @ np.asarray(W1)) * k1
    b1f = np.asarray(b1) * k1 + (bn1_b - bn1_m * k1)
    w1s = W1f.sum(0)                      # colsum for the LN -mu term
    k2 = bn2_g / np.sqrt(bn2_v + EPS)
    W2f = np.asarray(W2) * k2[None, :]
    b2f = np.asarray(b2) * k2 + (bn2_b - bn2_m * k2)
    k3 = bn3_g / np.sqrt(bn3_v + EPS)
    W3f = np.asarray(W3) * k3[None, :]
    b3f = np.asarray(b3) * k3 + (bn3_b - bn3_m * k3)

    # real edges only, assigned to dst owner core, bucketed by dst block
    core_of = dst // NPC
    dloc = dst - core_of * NPC
    # z-table rows are partition-major within a core's shard:
    # node local id n (block j = n//128, lane p = n%128) -> row p*NBLK + j
    nloc = src % NPC
    srcpad = (src // NPC) * NPAD + (nloc % P) * NBLK + (nloc // P)

    counts = np.zeros((NC, NBLK), np.int64)
    per_core = []
    for c in range(NC):
        m = core_of == c
        s = srcpad[m]
        d = dloc[m]
        o = np.argsort(d, kind="stable")
        s, d = s[o], d[o]
        cell = (d >> 7).astype(np.int64)
        counts[c] = np.bincount(cell, minlength=NBLK)
        per_core.append((s, d, cell))

    T = np.maximum(1, -(-counts.max(0) // P))      # tiles per block, shared
    tile_off = np.concatenate([[0], np.cumsum(T)]).astype(np.int64)
    ntiles = int(tile_off[-1])

    idx16_list, dstrel_list = [], []
    for c in range(NC):
        s, d, cell = per_core[c]
        start = np.searchsorted(cell, np.arange(NBLK))
        pos = np.arange(len(cell)) - start[cell]
        slot = tile_off[cell] * P + pos
        idx_lin = np.zeros(ntiles * P, np.int32)          # pad -> row BIAS
        rel_lin = np.full(ntiles * P, 999.0, np.float32)  # pad -> no match
        idx_lin[slot] = (s - BIAS).astype(np.int32)
        rel_lin[slot] = (d & 127).astype(np.float32)
        assert idx_lin.min() >= -32768 and idx_lin.max() <= 32767
        idx16 = idx_lin.reshape(ntiles * 8, 16).T.astype(np.int16)
        idx16 = np.tile(idx16, (8, 1))                    # [128, ntiles*8]
        dstrel = rel_lin.reshape(ntiles, P).T             # [128, ntiles]
        idx16_list.append(np.ascontiguousarray(idx16))
        dstrel_list.append(np.ascontiguousarray(_to_bf(dstrel)))

    # group blocks so one dma_gather stays under the Q7 scratch limit
    groups = []  # list of (b0, nb) block ranges
    b0 = 0
    while b0 < NBLK:
        nb = 0
        while (b0 + nb < NBLK
               and (tile_off[b0 + nb + 1] - tile_off[b0]) * P <= GIDX):
            nb += 1
        nb = max(nb, 1)
        groups.append((b0, nb))
        b0 += nb

    # per-core node data: x in bf16, node-major + feature-major(transposed)
    xbf = np.asarray(x, np.float32).astype(BF16)
    xp_list, xt_list, disb_list = [], [], []
    for c in range(NC):
        xp = np.zeros((NPAD, D), BF16)
        xp[:NPC] = xbf[c * NPC:(c + 1) * NPC]
        xpm = xp.reshape(NBLK, P, D).transpose(1, 0, 2).reshape(P, NBLK * D)
        xp_list.append(np.ascontiguousarray(xpm))
        xt = np.zeros((P, KD * NPAD), BF16)   # [128, kc*NPAD + node]
        xf = xp.reshape(NPAD, KD, P)          # node, chunk, feat
        xt[:] = np.transpose(xf, (2, 1, 0)).reshape(P, KD * NPAD)
        xt_list.append(np.ascontiguousarray(xt))
        db = np.ones(NPAD, np.float32)
        db[:NPC] = dis[c * NPC:(c + 1) * NPC]
        disb_list.append(np.ascontiguousarray(db.reshape(NBLK, P).T))

    iota = np.tile(np.arange(P, dtype=np.float32), (P, 16))
    ident = np.eye(P, dtype=np.float32)

    consts = dict(
        w1=_to_bf(W1f), w2=_to_bf(W2f), w3=_to_bf(W3f),
        fc1w=_to_bf(np.asarray(fc1_W)), fc2w=_to_bf(np.asarray(fc2_W)),
        w1s=_rep(w1s), zb1=_rep(zb1), b1f=_rep(b1f), b2f=_rep(b2f),
        b3f=_rep(b3f), fc1b=_rep(fc1_b), lncg=_rep(lnc_g), lncb=_rep(lnc_b),
        fc2b=_rep(fc2_b), iota=_to_bf(iota), idn=_to_bf(ident),
    )
    in_maps = []
    for c in range(NC):
        m = dict(consts)
        m.update(xp=xp_list[c], xt=xt_list[c], disb=disb_list[c],
                 idx16=idx16_list[c], dstrel=dstrel_list[c])
        in_maps.append(m)

    cfg = dict(N=N, D=D, E=E, H1=H1, H2=H2, H3=H3, HC=HC, C=C, NPC=NPC,
               NBLK=NBLK, NPAD=NPAD, NTAB=NTAB, BIAS=BIAS, ntiles=ntiles,
               T=T.tolist(), tile_off=tile_off.tolist(), groups=groups)
    return cfg, in_maps


def build_nc(cfg):
    stop = cfg.get("stop", "")
    D, H1, H2, H3 = cfg["D"], cfg["H1"], cfg["H2"], cfg["H3"]
    HC, C = cfg["HC"], cfg["C"]
    NBLK, NPAD, NTAB, BIAS = cfg["NBLK"], cfg["NPAD"], cfg["NTAB"], cfg["BIAS"]
    ntiles, T, tile_off = cfg["ntiles"], cfg["T"], cfg["tile_off"]
    groups = cfg["groups"]
    if cfg.get("gidx"):
        gmax = int(cfg["gidx"])
        groups = []
        b0 = 0
        while b0 < NBLK:
            nb = 0
            while (b0 + nb < NBLK
                   and (tile_off[b0 + nb + 1] - tile_off[b0]) * P <= gmax):
                nb += 1
            nb = max(nb, 1)
            groups.append((b0, nb))
            b0 += nb
    KD = D // P
    nq = int(cfg.get("nq", NQ))
    shared_ag = int(cfg.get("shared_ag", 1))
    f8 = int(cfg.get("f8", 1))
    F8 = mybir.dt.float8e4
    ZDT = F8 if f8 else BF          # z-table dtype
    TSL = 256 if f8 else TS         # elements per 256B table row

    nc = bacc.Bacc("TRN2", target_bir_lowering=False, debug=False,
                   num_devices=NC, num_swdge_queues=nq)
    dt = nc.dram_tensor
    ap_xp = dt("xp", [P, NBLK * D], BF, kind="ExternalInput").ap()
    ap_xt = dt("xt", [P, KD * NPAD], BF, kind="ExternalInput").ap()
    ap_disb = dt("disb", [P, NBLK], F32, kind="ExternalInput").ap()
    ap_idx16 = dt("idx16", [P, ntiles * 8], mybir.dt.int16,
                  kind="ExternalInput").ap()
    ap_dstrel = dt("dstrel", [P, ntiles], BF, kind="ExternalInput").ap()
    ap_w1 = dt("w1", [D, H1], BF, kind="ExternalInput").ap()
    ap_w2 = dt("w2", [H1, H2], BF, kind="ExternalInput").ap()
    ap_w3 = dt("w3", [H2, H3], BF, kind="ExternalInput").ap()
    ap_fc1w = dt("fc1w", [H3, HC], BF, kind="ExternalInput").ap()
    ap_fc2w = dt("fc2w", [HC, C], BF, kind="ExternalInput").ap()
    reps = {}
    for nm, wd in [("w1s", H1), ("zb1", H1), ("b1f", H1), ("b2f", H2),
                   ("b3f", H3), ("fc1b", HC), ("lncg", HC), ("lncb", HC),
                   ("fc2b", C)]:
        reps[nm] = dt(nm, [P, wd], F32, kind="ExternalInput").ap()
    ap_iota = dt("iota", [P, 16 * P], BF, kind="ExternalInput").ap()
    ap_idn = dt("idn", [P, P], BF, kind="ExternalInput").ap()
    ap_out = dt("out", [NPAD, C], F32, kind="ExternalOutput").ap()

    HH = [H1, H2, H3]

    with tile.TileContext(nc) as tc:
        with (
            tc.tile_pool(name="const", bufs=1) as cp,
            tc.tile_pool(name="stage", bufs=1) as st,
            tc.tile_pool(name="work", bufs=3) as wk,
            tc.tile_pool(name="small", bufs=4) as sm,
            tc.tile_pool(name="psA", bufs=3, space="PSUM") as psA,
            tc.tile_pool(name="psZ", bufs=2, space="PSUM") as psZ,
            tc.tile_pool(name="psT", bufs=2, space="PSUM") as psT,
            tc.tile_pool(name="dram", bufs=1, space="DRAM") as dram,
        ):
            # ---- constants to SBUF
            def load_const(ap, shape, dtype):
                t = cp.tile(shape, dtype, tag=f"c{ap.tensor.name}",
                            name=f"c{ap.tensor.name}")
                nc.sync.dma_start(t[:], ap)
                return t

            t_w1 = cp.tile([P, KD * H1], BF, tag="w1")
            nc.sync.dma_start(t_w1[:].rearrange("p (k h) -> p k h", h=H1),
                              ap_w1.rearrange("(k p) h -> p k h", p=P))
            t_w2 = load_const(ap_w2, [H1, H2], BF)
            t_w3 = load_const(ap_w3, [H2, H3], BF)
            t_fc1w = load_const(ap_fc1w, [H3, HC], BF)
            t_fc2w = load_const(ap_fc2w, [HC, C], BF)
            t_rep = {}
            for nm in reps:
                t_rep[nm] = load_const(reps[nm], list(reps[nm].shape), F32)
            t_iota = load_const(ap_iota, [P, 16 * P], BF)
            t_idn = load_const(ap_idn, [P, P], BF)
            t_disb = load_const(ap_disb, [P, NBLK], F32)
            t_eps = cp.tile([P, 1], F32, tag="eps")
            nc.vector.memset(t_eps[:], float(EPS))
            t_idx = cp.tile([P, ntiles * 8], mybir.dt.int16, tag="idx")
            nc.sync.dma_start(t_idx[:], ap_idx16)
            t_drel = cp.tile([P, ntiles], BF, tag="drel")
            nc.sync.dma_start(t_drel[:], ap_dstrel)

            z_local = [dram.tile([NPAD, TSL], ZDT, tag=f"zloc{l}",
                                 name=f"zloc{l}") for l in range(3)]
            z_full = [dram.tile([NTAB, TSL], ZDT, tag=f"zfull{l}",
                                name=f"zfull{l}",
                                addr_space="Shared" if shared_ag else "Local")
                      for l in range(3)]
            zs_buf = [st.tile([P, NBLK * HH[l]], BF, tag=f"zs{l}",
                              name=f"zs{l}") for l in range(3)]
            zs_v = [zs_buf[l][:].rearrange("p (j h) -> p j h", h=HH[l])
                    for l in range(3)]
            ztab = st.tile([P, NBLK * TSL], ZDT, tag="ztab", name="ztab")
            ztab_v = ztab[:].rearrange("p (j s) -> p j s", s=TSL)

            def write_ztable(l, Fo):
                """zs_buf[l] (tight bf16) -> padded table row dtype -> HBM."""
                nc.vector.tensor_copy(ztab_v[:, :, 0:Fo], zs_v[l])
                nc.sync.dma_start(
                    z_local[l][:].rearrange("(p j) s -> p (j s)", p=P),
                    ztab[:])
            out_buf = st.tile([P, NBLK * C], F32, tag="outb")

            # pre-allocate all persistent staging tiles so the xin pool
            # (phase-A inputs) sits on top of the stack and frees cleanly
            ystage = st.tile([P, NBLK * H1], BF, tag="ystage", name="ystage")
            t_hst = st.tile([P, NBLK * H1], F32, tag="hst", name="hst")
            t_hb = st.tile([P, NBLK * H1], BF, tag="hb", name="hb")
            t_zn = st.tile([P, NBLK * H2], F32, tag="zn", name="zn")
            t_z4 = st.tile([P, NBLK * HC], F32, tag="z4", name="z4")
            t_r4 = st.tile([P, NBLK * HC], BF, tag="r4", name="r4")

            def bcast_node(t, w):
                """[128, NBLK] tile -> broadcast AP [128, NBLK, w] (0-stride)."""
                a = t[:]
                return bass.AP(a.tensor, a.offset, a.ap + [[0, w]])

            def bcast_feat(t, w):
                """[128, w] tile -> broadcast AP [128, NBLK, w] (0-stride blk)."""
                a = t[:]
                return bass.AP(a.tensor, a.offset,
                               [a.ap[0], [0, NBLK], a.ap[1]])

            # ============ phase A: stats + z1 = LN(x) @ W1f (folded) ========
            if stop != "Z":
              with tc.tile_pool(name="xin", bufs=1) as xin:
                t_xp = xin.tile([P, NBLK * D], BF, tag="xp")
                nc.sync.dma_start(t_xp[:], ap_xp)
                t_xt = xin.tile([P, KD * NPAD], BF, tag="xt")
                nc.sync.dma_start(t_xt[:], ap_xt)
                t1 = xin.tile([P, NBLK * H1], BF, tag="t1", name="t1")
                t2 = xin.tile([P, NBLK * H1], BF, tag="t2", name="t2")

                # stats: mu, rstd per node (x^2 accum on ACT, sums on DVE)
                ssum = sm.tile([P, NBLK], F32, tag="ssum")
                nc.vector.reduce_sum(
                    ssum[:].rearrange("p (j o) -> p j o", o=1),
                    t_xp[:].rearrange("p (j d) -> p j d", d=D),
                    axis=mybir.AxisListType.X)
                s2 = sm.tile([P, NBLK], F32, tag="s2")
                sqscr = wk.tile([P, D], F32, tag="sqscr")
                for b in range(NBLK):
                    nc.scalar.activation(
                        sqscr[:], t_xp[:, b * D:(b + 1) * D],
                        mybir.ActivationFunctionType.Square,
                        accum_out=s2[:, b:b + 1])
                mu = sm.tile([P, NBLK], F32, tag="mu")
                nc.vector.tensor_scalar_mul(mu[:], ssum[:], 1.0 / D)
                musq = sm.tile([P, NBLK], F32, tag="musq")
                nc.vector.tensor_tensor(musq[:], mu[:], mu[:],
                                        op=mybir.AluOpType.mult)
                var = sm.tile([P, NBLK], F32, tag="var")
                nc.vector.tensor_scalar_mul(var[:], s2[:], 1.0 / D)
                nc.vector.tensor_tensor(var[:], var[:], musq[:],
                                        op=mybir.AluOpType.subtract)
                std = sm.tile([P, NBLK], F32, tag="std")
                nc.scalar.activation(std[:], var[:],
                                     mybir.ActivationFunctionType.Sqrt,
                                     bias=t_eps[:], scale=1.0)
                rstd = sm.tile([P, NBLK], F32, tag="rstd")
                nc.vector.reciprocal(rstd[:], std[:])
                # a = disb*rstd ; m2 = -disb*rstd*mu
                a_sc = sm.tile([P, NBLK], F32, tag="a_sc")
                nc.vector.tensor_tensor(a_sc[:], t_disb[:], rstd[:],
                                        op=mybir.AluOpType.mult)
                m2 = sm.tile([P, NBLK], F32, tag="m2")
                nc.vector.tensor_tensor(m2[:], a_sc[:], mu[:],
                                        op=mybir.AluOpType.mult)
                nc.vector.tensor_scalar_mul(m2[:], m2[:], -1.0)

                # y = x @ W1f + zb1 per block on PE (zb1 added via DVE below)
                for b in range(NBLK):
                    zp = psZ.tile([P, H1], F32, tag="zps")
                    for kc in range(KD):
                        nc.tensor.matmul(
                            zp[:],
                            lhsT=t_xt[:, kc * NPAD + b * P:
                                      kc * NPAD + (b + 1) * P],
                            rhs=t_w1[:, kc * H1:(kc + 1) * H1],
                            start=(kc == 0), stop=(kc == KD - 1))
                    nc.vector.tensor_copy(ystage[:, b * H1:(b + 1) * H1],
                                          zp[:])
                # zs0 = a*y + m2*w1s + disb*zb1   (batched, bf16 out)
                nc.vector.tensor_tensor(
                    t1[:].rearrange("p (j h) -> p j h", h=H1),
                    bcast_node(m2, H1), bcast_feat(t_rep["w1s"], H1),
                    op=mybir.AluOpType.mult)
                nc.vector.tensor_tensor(
                    t2[:].rearrange("p (j h) -> p j h", h=H1),
                    bcast_node(t_disb, H1), bcast_feat(t_rep["zb1"], H1),
                    op=mybir.AluOpType.mult)
                nc.vector.tensor_tensor(t1[:], t1[:], t2[:],
                                        op=mybir.AluOpType.add)
                nc.vector.tensor_tensor(
                    t2[:].rearrange("p (j h) -> p j h", h=H1),
                    ystage[:].rearrange("p (j h) -> p j h", h=H1),
                    bcast_node(a_sc, H1), op=mybir.AluOpType.mult)
                nc.vector.tensor_tensor(
                    zs_v[0],
                    t2[:].rearrange("p (j h) -> p j h", h=H1),
                    t1[:].rearrange("p (j h) -> p j h", h=H1),
                    op=mybir.AluOpType.add)
                write_ztable(0, H1)

            RP = int(cfg.get("R", 1))
            _gq = [0]

            def edge_layer(l, Fh, Fo, t_wnext, postbias, mode="full",
                           reps_=(1, 1, 1, 1), zf=None):
                rep_ag, rep_g, rep_oh, rep_mm = reps_
                if zf is None:
                    zf = z_full
                if cfg.get("no_cc"):
                    for c in range(NC):
                        nc.sync.dma_start(
                            zf[l][c * NPAD:(c + 1) * NPAD, :],
                            z_local[l][:])
                else:
                    for ra in range(rep_ag):
                        zdst = zf[l] if ra == 0 else dram.tile(
                            [NTAB, TSL], ZDT, tag=f"zfr{ra}", name=f"zfr{ra}",
                            addr_space="Shared" if shared_ag else "Local")
                        nc.gpsimd.collective_compute(
                            "AllGather", mybir.AluOpType.bypass,
                            replica_groups=[list(range(NC))],
                            ins=[z_local[l][:].opt()],
                            outs=[zdst[:].opt()],
                        )
                if mode == "ag":
                    return
                hstage = t_hst[:, 0:NBLK * Fh]
                for (b0, nb) in groups:
                    t0 = tile_off[b0]
                    t1_ = tile_off[b0 + nb]
                    gt = t1_ - t0
                    gbuf = gp.tile([P, gt * Fh], ZDT, tag="gbuf")
                    for _ in range(rep_g):
                        _gq[0] += 1
                        nc.gpsimd.dma_gather(
                            out_ap=gbuf[:].rearrange("p (n f) -> p n f", f=Fh),
                            in_ap=zf[l][BIAS:, 0:Fh],
                            idxs_ap=t_idx[:, t0 * 8:t1_ * 8],
                            num_idxs=gt * P,
                            num_idxs_reg=gt * P,
                            elem_size=Fh,
                            elem_step=TSL,
                            single_packet=False,
                            queue_num=_gq[0] % nq,
                        )
                    sbuf = op_.tile([P, gt * P], ZDT, tag="sbufS")
                    if mode == "gather0":
                        nc.vector.tensor_copy(out_buf[:, 0:C], gbuf[:, 0:C])
                        continue
                    for _ in range(rep_oh):
                        for s0 in range(0, gt, 16):
                            s1 = min(s0 + 16, gt)
                            dr = t_drel[:, t0 + s0:t0 + s1]
                            dr_b = bass.AP(dr.tensor, dr.offset,
                                           dr.ap + [[0, P]])
                            nc.vector.tensor_tensor(
                                out=sbuf[:, s0 * P:s1 * P].rearrange(
                                    "p (t w) -> p t w", w=P),
                                in0=t_iota[:, 0:(s1 - s0) * P].rearrange(
                                    "p (t w) -> p t w", w=P),
                                in1=dr_b,
                                op=mybir.AluOpType.is_equal)
                    if mode == "gather":
                        nc.vector.tensor_copy(out_buf[:, 0:C], gbuf[:, 0:C])
                        nc.vector.tensor_copy(out_buf[:, C:2 * C],
                                              sbuf[:, 0:C])
                        continue
                    for b in range(b0, b0 + nb):
                        agg = psA.tile([P, Fh], F32, tag="agg")
                        nt = T[b]
                        base = tile_off[b]
                        for _ in range(rep_mm):
                            for t in range(nt):
                                g = base + t - t0
                                nc.tensor.matmul(
                                    agg[:],
                                    lhsT=sbuf[:, g * P:(g + 1) * P],
                                    rhs=gbuf[:, g * Fh:(g + 1) * Fh],
                                    start=(t == 0), stop=(t == nt - 1))
                        nc.vector.tensor_copy(
                            hstage[:, b * Fh:(b + 1) * Fh], agg[:])
                if mode in ("gather0", "gather"):
                    return
                # epilogue (batched): h = relu(disb*(agg + zs) + bias)
                nc.vector.tensor_tensor(
                    hstage[:].rearrange("p (j h) -> p j h", h=Fh),
                    hstage[:].rearrange("p (j h) -> p j h", h=Fh),
                    zs_v[l], op=mybir.AluOpType.add)
                nc.vector.tensor_tensor(
                    hstage[:].rearrange("p (j h) -> p j h", h=Fh),
                    hstage[:].rearrange("p (j h) -> p j h", h=Fh),
                    bcast_node(t_disb, Fh), op=mybir.AluOpType.mult)
                nc.vector.tensor_tensor(
                    hstage[:].rearrange("p (j h) -> p j h", h=Fh),
                    hstage[:].rearrange("p (j h) -> p j h", h=Fh),
                    bcast_feat(postbias, Fh), op=mybir.AluOpType.add)
                hb = t_hb[:, 0:NBLK * Fh]
                nc.scalar.activation(hb[:], hstage[:],
                                     mybir.ActivationFunctionType.Relu)
                if t_wnext is None:
                    return hb
                # z_{l+1} = disb * (h @ Wnext) per block, batched scale
                znext = t_zn[:, 0:NBLK * Fo]
                for b in range(NBLK):
                    tp = psT.tile([P, P], BF, tag="tps")
                    nc.tensor.transpose(tp[0:Fh, :],
                                        hb[:, b * Fh:(b + 1) * Fh], t_idn[:])
                    hT = wk.tile([P, P], BF, tag="hT")
                    nc.vector.tensor_copy(hT[0:Fh, :], tp[0:Fh, :])
                    zp = psZ.tile([P, Fo], F32, tag="zps")
                    nc.tensor.matmul(zp[:], lhsT=hT[0:Fh, :], rhs=t_wnext[:],
                                     start=True, stop=True)
                    nc.vector.tensor_copy(znext[:, b * Fo:(b + 1) * Fo],
                                          zp[:])
                nc.vector.tensor_tensor(
                    zs_v[l + 1],
                    znext[:].rearrange("p (j h) -> p j h", h=Fo),
                    bcast_node(t_disb, Fo), op=mybir.AluOpType.mult)
                write_ztable(l + 1, Fo)
                return None

            def classifier(h3):
                # z4 = h3 @ fc1W + fc1b ; r = relu(LN(z4)) ; out = r@fc2W+fc2b
                z4 = t_z4
                for b in range(NBLK):
                    tp = psT.tile([P, P], BF, tag="tps")
                    nc.tensor.transpose(tp[0:H3, :],
                                        h3[:, b * H3:(b + 1) * H3], t_idn[:])
                    hT = wk.tile([P, P], BF, tag="hT")
                    nc.vector.tensor_copy(hT[0:H3, :], tp[0:H3, :])
                    zp = psZ.tile([P, HC], F32, tag="zps")
                    nc.tensor.matmul(zp[:], lhsT=hT[0:H3, :], rhs=t_fc1w[:],
                                     start=True, stop=True)
                    nc.vector.tensor_copy(z4[:, b * HC:(b + 1) * HC], zp[:])
                nc.vector.tensor_tensor(
                    z4[:].rearrange("p (j h) -> p j h", h=HC),
                    z4[:].rearrange("p (j h) -> p j h", h=HC),
                    bcast_feat(t_rep["fc1b"], HC), op=mybir.AluOpType.add)
                # LN over HC
                ssum = sm.tile([P, NBLK], F32, tag="ssum4")
                nc.vector.reduce_sum(
                    ssum[:].rearrange("p (j o) -> p j o", o=1),
                    z4[:].rearrange("p (j h) -> p j h", h=HC),
                    axis=mybir.AxisListType.X)
                mu = sm.tile([P, NBLK], F32, tag="mu4")
                nc.vector.tensor_scalar_mul(mu[:], ssum[:], 1.0 / HC)
                zc = wk.tile([P, NBLK * HC], F32, tag="zc")
                nc.vector.tensor_tensor(
                    zc[:].rearrange("p (j h) -> p j h", h=HC),
                    z4[:].rearrange("p (j h) -> p j h", h=HC),
                    bcast_node(mu, HC), op=mybir.AluOpType.subtract)
                zsq = wk.tile([P, NBLK * HC], F32, tag="zsq")
                nc.vector.tensor_tensor(zsq[:], zc[:], zc[:],
                                        op=mybir.AluOpType.mult)
                var = sm.tile([P, NBLK], F32, tag="var4")
                nc.vector.reduce_sum(
                    var[:].rearrange("p (j o) -> p j o", o=1),
                    zsq[:].rearrange("p (j h) -> p j h", h=HC),
                    axis=mybir.AxisListType.X)
                nc.vector.tensor_scalar_mul(var[:], var[:], 1.0 / HC)
                std = sm.tile([P, NBLK], F32, tag="std4")
                nc.scalar.activation(std[:], var[:],
                                     mybir.ActivationFunctionType.Sqrt,
                                     bias=t_eps[:], scale=1.0)
                rstd = sm.tile([P, NBLK], F32, tag="rstd4")
                nc.vector.reciprocal(rstd[:], std[:])
                nc.vector.tensor_tensor(
                    zc[:].rearrange("p (j h) -> p j h", h=HC),
                    zc[:].rearrange("p (j h) -> p j h", h=HC),
                    bcast_node(rstd, HC), op=mybir.AluOpType.mult)
                nc.vector.tensor_tensor(
                    zc[:].rearrange("p (j h) -> p j h", h=HC),
                    zc[:].rearrange("p (j h) -> p j h", h=HC),
                    bcast_feat(t_rep["lncg"], HC), op=mybir.AluOpType.mult)
                nc.vector.tensor_tensor(
                    zc[:].rearrange("p (j h) -> p j h", h=HC),
                    zc[:].rearrange("p (j h) -> p j h", h=HC),
                    bcast_feat(t_rep["lncb"], HC), op=mybir.AluOpType.add)
                r4 = t_r4
                nc.scalar.activation(r4[:], zc[:],
                                     mybir.ActivationFunctionType.Relu)
                for b in range(NBLK):
                    tp2 = psT.tile([P, P], BF, tag="tps")
                    nc.tensor.transpose(tp2[0:HC, :],
                                        r4[:, b * HC:(b + 1) * HC], t_idn[:])
                    rT = wk.tile([P, P], BF, tag="rT")
                    nc.vector.tensor_copy(rT[0:HC, :], tp2[0:HC, :])
                    op2 = psZ.tile([P, C], F32, tag="zps")
                    nc.tensor.matmul(op2[:], lhsT=rT[0:HC, :], rhs=t_fc2w[:],
                                     start=True, stop=True)
                    nc.vector.tensor_copy(out_buf[:, b * C:(b + 1) * C],
                                          op2[:])
                nc.vector.tensor_tensor(
                    out_buf[:].rearrange("p (j c) -> p j c", c=C),
                    out_buf[:].rearrange("p (j c) -> p j c", c=C),
                    bcast_feat(t_rep["fc2b"], C), op=mybir.AluOpType.add)

            gbn = int(cfg.get("gbufs", 4))
            with (
                tc.tile_pool(name="gath", bufs=gbn) as gp,
                tc.tile_pool(name="onehot", bufs=gbn) as op_,
            ):
                if stop == "Z":
                    nc.vector.memset(out_buf[:], 0.0)
                elif stop == "A":
                    nc.vector.memset(out_buf[:], 0.0)
                elif stop in ("AG", "G0", "G1", "L1"):
                    edge_layer(0, H1, H2, t_w2, t_rep["b1f"],
                               mode={"AG": "ag", "G0": "gather0",
                                     "G1": "gather", "L1": "full"}[stop])
                    nc.vector.memset(out_buf[:], 0.0)
                elif stop in ("AGR", "G0R", "G1R", "L1R"):
                    md = {"AGR": "ag", "G0R": "gather0", "G1R": "gather",
                          "L1R": "full"}[stop]
                    rp = {"AGR": (RP, 1, 1, 1), "G0R": (1, RP, 1, 1),
                          "G1R": (1, 1, RP, 1), "L1R": (1, 1, 1, RP)}[stop]
                    edge_layer(0, H1, H2, t_w2, t_rep["b1f"], mode=md,
                               reps_=rp)
                    nc.vector.memset(out_buf[:], 0.0)
                elif stop == "FR":
                    for r in range(RP):
                        zfr = z_full if r == 0 else [
                            dram.tile([NTAB, TSL], ZDT, tag=f"zfl{r}{l}",
                                      name=f"zfl{r}{l}",
                                      addr_space=("Shared" if shared_ag
                                                  else "Local"))
                            for l in range(3)]
                        edge_layer(0, H1, H2, t_w2, t_rep["b1f"], zf=zfr)
                        edge_layer(1, H2, H3, t_w3, t_rep["b2f"], zf=zfr)
                        h3 = edge_layer(2, H3, None, None, t_rep["b3f"],
                                        zf=zfr)
                        classifier(h3)
                else:
                    edge_layer(0, H1, H2, t_w2, t_rep["b1f"])
                    edge_layer(1, H2, H3, t_w3, t_rep["b2f"])
                    h3 = edge_layer(2, H3, None, None, t_rep["b3f"])
                    classifier(h3)

            nc.sync.dma_start(
                ap_out.rearrange("(j p) c -> p j c", p=P),
                out_buf[:].rearrange("p (j c) -> p j c", c=C))
    nc.compile()
    return nc


_CACHE = {}


def _get_nc(cfg):
    key = repr(sorted((k, str(v)) for k, v in cfg.items()))
    if key not in _CACHE:
        _CACHE[key] = build_nc(cfg)
    return _CACHE[key]


def kernel(**inputs):
    cfg, in_maps = preprocess(**inputs)
    nc = _get_nc(cfg)
    res = bass_utils.run_bass_kernel_spmd(nc, in_maps, core_ids=list(range(NC)))
    NPC, NPAD, N, C = cfg["NPC"], cfg["NPAD"], cfg["N"], cfg["C"]
    out = np.empty((N, C), np.float32)
    for c in range(NC):
        out[c * NPC:(c + 1) * NPC] = res.results[c]["out"][:NPC]
    return out
